# revision 32
# baseline (speedup 1.0000x reference)
"""Trainium2 Bass kernel for nn_KTM_71339406786898.

Fused dual-input attention block (see reference.py), data-parallel over
batch B=8 across 8 NeuronCores, one batch element per core.

v2: the energy computation is refactored via G = Wq^T Wk so that each
energy matmul is a single full-width K=33 fp8 matmul per j-tile
(instead of K=16 4-row-banded).  This quadruples PE-cycles per energy
and matches PE throughput to the exp (ACT/DVE) wall, keeping the PE
array streaming densely back-to-back.  That keeps the PE_HAM activity
monitor in the un-throttled state (2.4 GHz); the v1 phase structure
stalled the PE every granule on exp, pinning the clock at 1.2 GHz for
the whole kernel (every matmul cost exactly 2x).

Pipeline (one chunk lag between energy+exp and apply):
  iteration it (0..8):
    g = 0..15:  E(it,g): 2 MMs [33,128]x[33,512] -> e_ps (PSUM)
                exp(it,g): ACT Exp (bias-shifted) or DVE Schraudolph
                           int8 bit-trick -> e_sb fp8e5 (queue of 33)
                A(it-1,g): DoubleRow fp8 apply acc += [v|1]^T e
                g==0: conv_chunk(it-2) (9 MMs K=96 + fused wo@w_1)
    normalize(it-1): recip bit-trick + gpsimd broadcast; z = acc*r + x
    stage zr dy-shifted conv planes (DMA)

Host folds: G=Wq^T Wk into one [65,33] projection, biases via ones
rows, gamma into wv, bn_s into conv weights, wo@w{2,3}_1 fused.
"""

import sys

import ml_dtypes
import numpy as np

for _p in ("/opt/trn_rl_repo", "/root/.axon_site/_ro/trn_rl_repo"):
    if _p not in sys.path:
        sys.path.append(_p)

import concourse.bass as bass
import concourse.mybir as mybir
import concourse.tile as tile
from concourse import bacc
from concourse.bass_utils import run_bass_kernel_spmd

B, C, H, W = 8, 32, 64, 64
CQ = C // 2
HW = H * W
NCORES = 8

IC = 512             # i-chunk width
NCH = HW // IC       # 8 chunks
JT = 128             # j-tile (partitions)
NJT = HW // JT       # 32 j-tiles
NGR = NJT // 2       # 16 granules of 2 j-tiles
PW = W + 2           # padded conv width (66)
PHW = PW * (H + 2)   # padded conv plane
RPC = IC // W        # spatial rows per chunk (8)
KE = 2 * C + 1       # energy contraction, zero-padded to 65 rows
KER = C + 1          # real contraction rows (32 ch + 1 aug)

F32 = mybir.dt.float32
F32R = mybir.dt.float32r
I32 = mybir.dt.int32
I8 = mybir.dt.int8
F8E4 = mybir.dt.float8e4
F8E5 = mybir.dt.float8e5
AF = mybir.ActivationFunctionType
ALU = mybir.AluOpType
DR = mybir.MatmulPerfMode.DoubleRow

# Schraudolph e5m2: i8 = E*4*log2(e) + SCH_C, bitcast -> 2^((i-60)/4)
# ~= e^E * 2^((SCH_C-60)/4).  ACT path matches via exp-input bias
# ln(2)*(SCH_C-60)/4.
SCH_C = 63.0
SCH_SLOPE = float(4.0 / np.log(2.0))
ACT_BIAS = float(np.log(2.0) * (SCH_C - 60.0) / 4.0)
RECIP_MAGIC = 0x7EF312AC

# exp engine schedule per chunk: one entry per granule, 'A'=ACT 'D'=DVE
ESCHED = "ADADADADADADADAD"
assert len(ESCHED) == NGR


def _r(ap):
    return ap.bitcast(F32R)


def build_program():
    nc = bacc.Bacc("TRN2", target_bir_lowering=False, debug=False)

    x2d = nc.dram_tensor("x2", [C, HW], F32R, kind="ExternalInput").ap()
    x3d = nc.dram_tensor("x3", [C, HW], F32R, kind="ExternalInput").ap()
    onesd = nc.dram_tensor("ones", [1, HW], F32R, kind="ExternalInput").ap()
    wpd = nc.dram_tensor("wp", [C * 2 + 1, KE], F32R, kind="ExternalInput").ap()
    wvad = nc.dram_tensor("wva", [C * 2 + 1, 2 * C + 2], F32R, kind="ExternalInput").ap()
    wcsd = nc.dram_tensor("wcs", [3 * C, 6 * C], F32R, kind="ExternalInput").ap()
    bbd = nc.dram_tensor("bb", [2 * C, 1], F32, kind="ExternalInput").ap()
    wabd = nc.dram_tensor("wab", [KE, 2 * C], F32R, kind="ExternalInput").ap()
    bfind = nc.dram_tensor("bfin", [C, 1], F32, kind="ExternalInput").ap()
    outd = nc.dram_tensor("out", [C, HW], F32, kind="ExternalOutput").ap()

    with tile.TileContext(nc) as tc:
        _emit(nc, tc, x2d, x3d, onesd, wpd, wvad, wcsd, bbd,
              wabd, bfind, outd)
    nc.compile()
    return nc


def _emit(nc, tc, x2d, x3d, onesd, wpd, wvad, wcsd, bbd,
          wabd, bfind, outd):
    from contextlib import ExitStack

    ctx = ExitStack()
    with ctx:
        consts = ctx.enter_context(tc.tile_pool(name="consts", bufs=1))
        xp = ctx.enter_context(tc.tile_pool(name="xp", bufs=1))
        qk = ctx.enter_context(tc.tile_pool(name="qk", bufs=1))
        vs = ctx.enter_context(tc.tile_pool(name="vs", bufs=1))
        es = ctx.enter_context(tc.tile_pool(name="es", bufs=33))
        zs = ctx.enter_context(tc.tile_pool(name="zs", bufs=2))
        outp = ctx.enter_context(tc.tile_pool(name="outp", bufs=2))
        zrp = ctx.enter_context(tc.tile_pool(name="zrp", bufs=1))
        ep = ctx.enter_context(tc.tile_pool(name="ep", bufs=2, space="PSUM"))
        accp = ctx.enter_context(tc.tile_pool(name="accp", bufs=2, space="PSUM"))
        cvp = ctx.enter_context(tc.tile_pool(name="cvp", bufs=2, space="PSUM"))

        # ---- constant loads -------------------------------------------------
        wp = consts.tile([C * 2 + 1, KE], F32R, tag="wp")
        nc.sync.dma_start(out=wp[:], in_=wpd)
        wva = consts.tile([C * 2 + 1, 2 * C + 2], F32R, tag="wva")
        nc.sync.dma_start(out=wva[:], in_=wvad)
        wcs = consts.tile([3 * C, 6 * C], F32R, tag="wcs")
        nc.sync.dma_start(out=wcs[:], in_=wcsd)
        bb2 = consts.tile([2 * C, 1], F32, tag="bb")
        nc.sync.dma_start(out=bb2[:], in_=bbd)
        wab2 = consts.tile([KE, 2 * C], F32R, tag="wab")
        nc.sync.dma_start(out=wab2[:], in_=wabd)
        bfin = consts.tile([C, 1], F32, tag="bfin")
        nc.sync.dma_start(out=bfin[:], in_=bfind)
        ebias = consts.tile([JT, 1], F32, tag="ebias")
        nc.gpsimd.memset(ebias[:], ACT_BIAS)
        # warm the ACT exp table set while input DMAs run
        tblw = consts.tile([1, 1], F32, tag="tblw")
        nc.scalar.activation(tblw[:], ebias[0:1, 0:1], AF.Exp)
        # warm the PE clock during the input-DMA wait (dummy matmuls on
        # the already-resident weight tile; outputs discarded)
        wdum = consts.tile([KE, IC], F32R, tag="wdum")
        nc.gpsimd.memset(wdum[:].bitcast(F32), 0.0)
        for _w in range(16):
            wps = ep.tile([JT, 2 * IC], F32, tag="e")
            nc.tensor.matmul(wps[0:KE, 0:IC], wp[:], wdum[:],
                             start=True, stop=True)

        # ---- inputs (split loads across queues for overlap) ----------------
        HB = HW // 2
        xcat = xp.tile([2 * C + 1, HW], F32R, tag="xcat")
        x3b = xp.tile([C, HW], F32, tag="x3b")
        nc.sync.dma_start(out=xcat[2 * C:2 * C + 1, :], in_=onesd)
        nc.sync.dma_start(out=xcat[0:C, 0:HB], in_=x2d[:, 0:HB])
        nc.sync.dma_start(out=xcat[0:C, HB:HW], in_=x2d[:, HB:HW])
        nc.scalar.dma_start(out=xcat[C:2 * C, 0:HB], in_=x3d[:, 0:HB])
        nc.scalar.dma_start(out=xcat[C:2 * C, HB:HW], in_=x3d[:, HB:HW])
        nc.gpsimd.dma_start(out=x3b[:, 0:HB], in_=x3d.bitcast(F32)[:, 0:HB])
        nc.gpsimd.dma_start(out=x3b[:, HB:HW], in_=x3d.bitcast(F32)[:, HB:HW])

        # ---- P/xm projections into fp8 --------------------------------------
        # p8[a, i] = fp8(W_P^T xcat[:, i]) : [33, HW]
        # xm8[a, j] = fp8([x2*x3; 1][:, j]) : [33, HW]
        # normal-mode K=65 zero-padded layout.  The whole main-loop PE
        # stream stays in ONE weight-path mode (normal, K>=65): mixing
        # DoubleRow and normal matmuls, or K<=64 rows, drops the PE to
        # half rate on HW.
        p8 = qk.tile([KE, HW], F8E4, tag="p8")
        xm8 = qk.tile([KE, HW], F8E4, tag="xm8")
        nc.gpsimd.memset(xm8[C:2 * C, :], 0.0)
        nc.gpsimd.memset(xm8[2 * C:KE, :], 0.0)
        nc.gpsimd.memset(xm8[C:KER, :], 1.0)

        for blk in range(NCH):
            sl = slice(blk * IC, (blk + 1) * IC)
            nc.vector.tensor_mul(xm8[0:C, sl],
                                 xcat[0:C, sl].bitcast(F32),
                                 x3b[:, sl])
            pp = ep.tile([JT, 2 * IC], F32, tag="e")
            nc.tensor.matmul(pp[0:KE, 0:IC], wp[:], xcat[:, sl],
                             start=True, stop=True)
            if blk % 2 == 0:
                nc.scalar.activation(p8[:, sl], pp[0:KE, 0:IC], AF.Copy)
            else:
                nc.vector.tensor_copy(out=p8[:, sl], in_=pp[0:KE, 0:IC])

        # ---- v^T stack: vstk[j_local, jt, c] = v2|v3|ones (fp8e4) ----------
        VM = 2 * C + 2   # 66 v columns (v2|v3|ones|pad)
        VST = 80         # j-tile stride in vstk (%16==0 for dual-fp8 LW)
        vstk = vs.tile([JT, NJT, VST], F8E4, tag="vstk")
        nc.gpsimd.memset(vstk[:, :, VM:VST], 0.0)
        VPB = 7  # v-tiles per psum half-bank

        def emit_vstack(jt, n_here):
            vt = ep.tile([JT, 2 * IC], F32, tag="e")
            for t in range(n_here):
                off = (t // VPB) * IC + (t % VPB) * VM
                nc.tensor.matmul(
                    vt[:, off:off + VM],
                    xcat[:, (jt + t) * JT:(jt + t + 1) * JT],
                    wva[:],
                    start=True, stop=True,
                )
            for half in range(2):
                cnt = min(VPB, n_here - half * VPB)
                if cnt <= 0:
                    break
                nc.scalar.activation(
                    vstk[:, jt + half * VPB:jt + half * VPB + cnt, 0:VM],
                    vt[:, half * IC:half * IC + cnt * VM]
                    .rearrange("p (t c) -> p t c", c=VM),
                    AF.Copy,
                )

        # static rstk, contraction rows C..KE zero-padded once
        rstk = vs.tile([KE, 2, IC], F32R, tag="rstk")
        nc.gpsimd.memset(rstk[C:2 * C, :, :].bitcast(F32), 0.0)
        nc.gpsimd.memset(rstk[2 * C:KE, :, :].bitcast(F32), 0.0)

        # ---- padded conv plane: z2 rows 0-31, z3 rows 32-63 ----------------
        z23p = zs.tile([2 * C, PHW], F32R, tag="z23p")
        z3d_ = z23p.rearrange("p (h w) -> p h w", h=H + 2, w=PW)
        nc.gpsimd.memset(z3d_[:, 0:1, :].bitcast(F32), 0.0)
        nc.gpsimd.memset(z3d_[:, H + 1:H + 2, :].bitcast(F32), 0.0)
        nc.gpsimd.memset(z3d_[:, 1:H + 1, 0:1].bitcast(F32), 0.0)
        nc.gpsimd.memset(z3d_[:, 1:H + 1, PW - 1:PW].bitcast(F32), 0.0)

        # ---- zR staging: 3 dy-shifted copies of the z plane rows -----------
        ZRL = 544
        zr2 = zrp.tile([3 * C, 2, ZRL], F32R, tag="zr2")
        zr3 = zrp.tile([3 * C, 2, ZRL], F32R, tag="zr3")

        def emit_zr(n, gs):
            src0 = RPC * n * PW
            for g in gs:
                eng = nc.sync if g % 2 == 0 else nc.gpsimd
                eng.dma_start(
                    out=zr2[32 * g:32 * g + C, n % 2, 0:RPC * PW],
                    in_=z23p[0:C, src0 + g * PW:src0 + g * PW + RPC * PW])
                eng.dma_start(
                    out=zr3[32 * g:32 * g + C, n % 2, 0:RPC * PW],
                    in_=z23p[C:2 * C, src0 + g * PW:src0 + g * PW + RPC * PW])

        def emit_zr2_partial(n):
            # band g=2 rows 0..6 (plane rows 8n+2 .. 8n+8): ready right
            # after normalize(n)
            s0 = (RPC * n + 2) * PW
            nc.sync.dma_start(out=zr2[64:64 + C, n % 2, 0:7 * PW],
                              in_=z23p[0:C, s0:s0 + 7 * PW])
            nc.gpsimd.dma_start(out=zr3[64:64 + C, n % 2, 0:7 * PW],
                                in_=z23p[C:2 * C, s0:s0 + 7 * PW])

        def emit_zr2_topup(n):
            # band g=2 last row (plane row 8n+9): needs normalize(n+1)
            s0 = (RPC * n + 9) * PW
            nc.sync.dma_start(out=zr2[64:64 + C, n % 2, 7 * PW:8 * PW],
                              in_=z23p[0:C, s0:s0 + PW])
            nc.gpsimd.dma_start(out=zr3[64:64 + C, n % 2, 7 * PW:8 * PW],
                                in_=z23p[C:2 * C, s0:s0 + PW])

        # ---- conv of chunk n (needs zr slices staged) ----------------------
        def conv_chunk(n):
            wb = n % 2
            cps2 = cvp.tile([C, IC], F32, tag="cv")
            cps3 = cvp.tile([C, IC], F32, tag="cv")
            for br, (zr, cp) in enumerate(((zr2, cps2), (zr3, cps3))):
                for dx in range(3):
                    nc.tensor.matmul(
                        cp[:],
                        wcs[:, (br * 3 + dx) * C:(br * 3 + dx + 1) * C],
                        zr[:, wb, dx:dx + RPC * PW]
                        .rearrange("p (r w) -> p r w", r=RPC, w=PW)[:, :, 0:W],
                        start=(dx == 0), stop=(dx == 2),
                    )
            for br, cp in enumerate((cps2, cps3)):
                # relu(conv + bn_b) on ACT (bias is per-partition)
                nc.scalar.activation(
                    rstk[0:C, br, :],
                    cp[:],
                    AF.Relu, bias=bb2[br * C:(br + 1) * C, 0:1],
                )
            op = cvp.tile([C, IC], F32, tag="cv")
            for br in range(2):
                nc.tensor.matmul(op[:], wab2[:, br * C:(br + 1) * C],
                                 rstk[:, br, :],
                                 start=(br == 0), stop=(br == 1))
            ob = outp.tile([C, IC], F32, tag="ob")
            nc.scalar.activation(ob[:], op[:], AF.Identity,
                                 bias=bfin[:, 0:1])
            nc.sync.dma_start(out=outd[:, n * IC:(n + 1) * IC], in_=ob[:])

        # ---- normalize chunk n: z = acc/denom + x --------------------------
        def normalize_cols(n, acc, c0, c1, tag):
            w = c1 - c0
            nr = w // W
            r_sb = zs.tile([1, IC], I32, tag="r7_" + tag, name="r7")
            nc.vector.tensor_scalar(
                out=r_sb[:, 0:w], in0=acc[2 * C:2 * C + 1, c0:c1].bitcast(I32),
                scalar1=-1, scalar2=RECIP_MAGIC,
                op0=ALU.mult, op1=ALU.add,
            )
            rbc = zs.tile([2 * C, IC], F32, tag="rb7_" + tag, name="rb7")
            nc.gpsimd.partition_broadcast(rbc[:, 0:w], r_sb.bitcast(F32)[:, 0:w])
            z23t = zs.tile([2 * C, IC], F32, tag="z7_" + tag, name="z7")
            nc.vector.tensor_mul(z23t[:, 0:w], acc[0:2 * C, c0:c1], rbc[:, 0:w])
            r0 = RPC * n + c0 // W
            nc.vector.tensor_add(
                z3d_[0:2 * C, 1 + r0:1 + r0 + nr, 1:1 + W],
                z23t[:, 0:w].rearrange("p (a b) -> p a b", a=nr, b=W),
                xcat[0:2 * C, n * IC + c0:n * IC + c1].bitcast(F32)
                .rearrange("p (a b) -> p a b", a=nr, b=W),
            )

        def normalize(n, acc):
            r_sb = zs.tile([1, IC], I32, tag="r_sb")
            nc.vector.tensor_scalar(
                out=r_sb[:], in0=acc[2 * C:2 * C + 1, :].bitcast(I32),
                scalar1=-1, scalar2=RECIP_MAGIC,
                op0=ALU.mult, op1=ALU.add,
            )
            rbc = zs.tile([2 * C, IC], F32, tag="rbc")
            nc.gpsimd.partition_broadcast(rbc[:], r_sb.bitcast(F32)[:])
            z23t = zs.tile([2 * C, IC], F32, tag="z23t")
            nc.vector.tensor_mul(z23t[:], acc[0:2 * C, :], rbc[:])
            r0 = RPC * n
            nc.vector.tensor_add(
                z3d_[0:2 * C, 1 + r0:1 + r0 + RPC, 1:1 + W],
                z23t[:, :].rearrange("p (a b) -> p a b", a=RPC, b=W),
                xcat[0:2 * C, n * IC:(n + 1) * IC].bitcast(F32)
                .rearrange("p (a b) -> p a b", a=RPC, b=W),
            )

        # ---- main loop: E/exp(it) + apply(it-1) + conv(it-2) ---------------
        esq = {}          # (chunk, granule) -> e_sb tile
        accs = {}         # chunk -> acc psum tile
        for it in range(NCH + 1):
            if it < NCH:
                accs[it] = accp.tile([2 * C + 2, IC], F32, tag="acc", name="acc")
            i0 = it * IC
            for g in range(NGR):
                # energies FIRST: at exp(g-2) completion the highest-
                # priority ready PE work must be E(g), so exp(g) issues
                # after only 432ns; the applies then fill the exp window
                if it < NCH:
                    e_ps = ep.tile([JT, 2 * IC], F32, tag="e")
                    for t in range(2):
                        jt_ = 2 * g + t
                        nc.tensor.matmul(
                            e_ps[:, t * IC:(t + 1) * IC],
                            xm8[:, jt_ * JT:(jt_ + 1) * JT],
                            p8[:, i0:i0 + IC],
                            start=True, stop=True,
                        )
                    e_sb = es.tile([JT, 2, IC], F8E5, tag="esb")
                    if ESCHED[g] == 'A':
                        nc.scalar.activation(
                            e_sb[:, :, :],
                            e_ps[:].rearrange("p (t n) -> p t n", t=2),
                            AF.Exp, bias=ebias[:, 0:1],
                        )
                    else:
                        nc.vector.tensor_scalar(
                            out=e_sb.bitcast(I8)[:, :, :],
                            in0=e_ps[:].rearrange("p (t n) -> p t n", t=2),
                            scalar1=SCH_SLOPE, scalar2=SCH_C,
                            op0=ALU.mult, op1=ALU.add,
                        )
                    esq[(it, g)] = e_sb
                if it == 0 and g in (3, 7, 11):
                    jt0 = {3: 0, 7: 14, 11: 28}[g]
                    emit_vstack(jt0, min(14, NJT - jt0))
            for g in range(NGR):
                if it >= 1:
                    e_prev = esq.pop((it - 1, g))
                    for t in range(2):
                        nc.tensor.matmul(
                            accs[it - 1][:],
                            vstk[:, 2 * g + t, 0:VM],
                            e_prev[:, t, :],
                            start=(g == 0 and t == 0),
                            stop=(g == NGR - 1 and t == 1),
                        )
            if 1 <= it < NCH:
                normalize(it - 1, accs.pop(it - 1))
                emit_zr(it - 1, (0, 1))
                emit_zr2_partial(it - 1)
                if it >= 2:
                    emit_zr2_topup(it - 2)
                    conv_chunk(it - 2)
        # tail: only chunk 7's normalize + convs 6,7 remain.  Normalize
        # the first spatial row of chunk 7 first so conv(6)'s topup (one
        # plane row) and matmuls can start while the rest normalizes.
        acc7 = accs.pop(NCH - 1)
        for dmy in range(14):
            e_ps = ep.tile([JT, 2 * IC], F32, tag="e")
            nc.tensor.matmul(e_ps[:, 0:IC],
                             xm8[:, (dmy % 8) * JT:(dmy % 8) * JT + JT],
                             p8[:, 0:IC], start=True, stop=True)
        normalize_cols(NCH - 1, acc7, 0, W, "a")
        s0 = (RPC * (NCH - 2) + 9) * PW
        nc.scalar.dma_start(out=zr2[64:64 + C, (NCH - 2) % 2, 7 * PW:8 * PW],
                            in_=z23p[0:C, s0:s0 + PW])
        nc.scalar.dma_start(out=zr3[64:64 + C, (NCH - 2) % 2, 7 * PW:8 * PW],
                            in_=z23p[C:2 * C, s0:s0 + PW])
        conv_chunk(NCH - 2)
        normalize_cols(NCH - 1, acc7, W, IC, "b")
        emit_zr(NCH - 1, (0, 1))
        emit_zr2_partial(NCH - 1)
        emit_zr2_topup(NCH - 1)
        conv_chunk(NCH - 1)


def prepare_params(wq, bq, wk, bk, wv2, bv2, wv3, bv3, gamma2, gamma3,
                   w2_3, bn2_s, bn2_b, w2_1, b2_1,
                   w3_3, bn3_s, bn3_b, w3_1, b3_1, wo, bo):
    f = np.float32
    wq, bq, wk, bk = (np.asarray(a, f) for a in (wq, bq, wk, bk))
    wv2, bv2, wv3, bv3 = (np.asarray(a, f) for a in (wv2, bv2, wv3, bv3))
    w2_3, bn2_s, bn2_b = (np.asarray(a, f) for a in (w2_3, bn2_s, bn2_b))
    w3_3, bn3_s, bn3_b = (np.asarray(a, f) for a in (w3_3, bn3_s, bn3_b))
    w2_1, b2_1, w3_1, b3_1 = (np.asarray(a, f) for a in (w2_1, b2_1, w3_1, b3_1))
    wo, bo = np.asarray(wo, f), np.asarray(bo, f)
    g2 = f(np.asarray(gamma2).reshape(-1)[0])
    g3 = f(np.asarray(gamma3).reshape(-1)[0])

    # energy fold: E[i,j] = P_aug[:,i] . xm_aug[:,j]
    #   P_aug = W_P^T @ [x2; x3; 1],  xm_aug = [x2*x3; 1]
    A = wq.T @ wk                    # [32, 32]
    u = wk.T @ bq                    # [32]
    v = wq.T @ bk                    # [32]
    s = float(bq @ bk)
    wp = np.zeros((2 * C + 1, KE), f)   # cols C+1..KE stay zero (K pad)
    wp[0:C, 0:C] = A
    wp[C:2 * C, 0:C] = A
    wp[2 * C, 0:C] = u
    wp[0:C, C] = v
    wp[C:2 * C, C] = v
    wp[2 * C, C] = s

    wva = np.zeros((2 * C + 1, 2 * C + 2), f)
    wva[0:C, 0:C] = wv2.T * g2
    wva[C:2 * C, C:2 * C] = wv3.T * g3
    wva[2 * C, 0:C] = bv2 * g2
    wva[2 * C, C:2 * C] = bv3 * g3
    wva[2 * C, 2 * C] = 1.0

    wcs = np.zeros((3 * C, 6 * C), f)
    for (br, w3x3, bns) in ((0, w2_3, bn2_s), (1, w3_3, bn3_s)):
        ws = w3x3 * bns[:, None, None, None]  # [o, ci, dy, dx]
        for dy in range(3):
            for dx in range(3):
                wcs[C * dy:C * (dy + 1),
                    (br * 3 + dx) * C:(br * 3 + dx + 1) * C] = \
                    ws[:, :, dy, dx].T

    bb = np.concatenate([bn2_b, bn3_b]).reshape(2 * C, 1).astype(f)
    wab = np.zeros((KE, 2 * C), f)      # rows C..KE stay zero (K pad)
    wab[0:C, 0:C] = (wo @ w2_1).T
    wab[0:C, C:] = (wo @ w3_1).T
    bfin = (wo @ (b2_1 + b3_1) + bo).astype(f).reshape(C, 1)

    return {
        "wp": wp, "wva": wva, "wcs": wcs, "bb": bb,
        "wab": wab, "bfin": bfin,
        "ones": np.ones((1, HW), f),
    }


_CACHED = {}


def _get_program():
    if "nc" not in _CACHED:
        _CACHED["nc"] = build_program()
    return _CACHED["nc"]


def make_in_maps(x2, x3, params):
    x2 = np.ascontiguousarray(np.asarray(x2, np.float32).reshape(B, C, HW))
    x3 = np.ascontiguousarray(np.asarray(x3, np.float32).reshape(B, C, HW))
    return [
        {"x2": x2[b], "x3": x3[b], **params}
        for b in range(NCORES)
    ]


def kernel(x2, x3, **kw):
    params = prepare_params(**kw)
    nc = _get_program()
    in_maps = make_in_maps(x2, x3, params)
    res = run_bass_kernel_spmd(nc, in_maps, list(range(NCORES)))
    out = np.stack([res.results[b]["out"].reshape(C, H, W)
                    for b in range(NCORES)])
    return out.astype(np.float32)


def _ensure_ntff_hook():
    import contextlib
    import ctypes
    import types

    if "antenv.axon_hooks" in sys.modules:
        return
    so_path = "/opt/axon/libaxon_pjrt.so"
    lib = ctypes.CDLL(so_path)
    lib.axon_start_nrt_profile.argtypes = [
        ctypes.POINTER(ctypes.c_int64), ctypes.c_size_t]
    lib.axon_start_nrt_profile.restype = ctypes.c_int64
    lib.axon_stop_nrt_profile.argtypes = [ctypes.c_char_p]
    lib.axon_stop_nrt_profile.restype = ctypes.c_int64

    @contextlib.contextmanager
    def _hook(output_dir, device_ids):
        import jax
        jax.devices()
        if device_ids:
            ids = (ctypes.c_int64 * len(device_ids))(*device_ids)
            rc = lib.axon_start_nrt_profile(ids, len(device_ids))
        else:
            rc = lib.axon_start_nrt_profile(None, 0)
        if rc != 0:
            raise RuntimeError(f"axon_start_nrt_profile rc={rc}")
        try:
            yield
        finally:
            n = lib.axon_stop_nrt_profile(str(output_dir).encode())
            if n < 0:
                raise RuntimeError(f"axon_stop_nrt_profile rc={n}")
            if n == 0:
                print("WARNING: NTFF capture wrote 0 files")

    mod = types.ModuleType("antenv.axon_hooks")
    mod.get_axon_ntff_profile_hook = lambda: _hook
    mod.set_axon_ntff_profile_hook = lambda h: None
    sys.modules["antenv.axon_hooks"] = mod


def run_traced(x2, x3, trace_cores=None, **kw):
    _ensure_ntff_hook()
    params = prepare_params(**kw)
    nc = _get_program()
    in_maps = make_in_maps(x2, x3, params)
    res = run_bass_kernel_spmd(nc, in_maps, list(range(NCORES)),
                               trace=True, trace_cores=trace_cores)
    out = np.stack([res.results[b]["out"].reshape(C, H, W)
                    for b in range(NCORES)])
    return out.astype(np.float32), res


# revision 34
# speedup vs baseline: 1.0033x; 1.0033x over previous
"""Trainium2 Bass kernel for nn_KTM_71339406786898.

Fused dual-input attention block (see reference.py), data-parallel over
batch B=8 across 8 NeuronCores, one batch element per core.

v2: the energy computation is refactored via G = Wq^T Wk so that each
energy matmul is a single full-width K=33 fp8 matmul per j-tile
(instead of K=16 4-row-banded).  This quadruples PE-cycles per energy
and matches PE throughput to the exp (ACT/DVE) wall, keeping the PE
array streaming densely back-to-back.  That keeps the PE_HAM activity
monitor in the un-throttled state (2.4 GHz); the v1 phase structure
stalled the PE every granule on exp, pinning the clock at 1.2 GHz for
the whole kernel (every matmul cost exactly 2x).

Pipeline (one chunk lag between energy+exp and apply):
  iteration it (0..8):
    g = 0..15:  E(it,g): 2 MMs [33,128]x[33,512] -> e_ps (PSUM)
                exp(it,g): ACT Exp (bias-shifted) or DVE Schraudolph
                           int8 bit-trick -> e_sb fp8e5 (queue of 33)
                A(it-1,g): DoubleRow fp8 apply acc += [v|1]^T e
                g==0: conv_chunk(it-2) (9 MMs K=96 + fused wo@w_1)
    normalize(it-1): recip bit-trick + gpsimd broadcast; z = acc*r + x
    stage zr dy-shifted conv planes (DMA)

Host folds: G=Wq^T Wk into one [65,33] projection, biases via ones
rows, gamma into wv, bn_s into conv weights, wo@w{2,3}_1 fused.
"""

import sys

import ml_dtypes
import numpy as np

for _p in ("/opt/trn_rl_repo", "/root/.axon_site/_ro/trn_rl_repo"):
    if _p not in sys.path:
        sys.path.append(_p)

import concourse.bass as bass
import concourse.mybir as mybir
import concourse.tile as tile
from concourse import bacc
from concourse.bass_utils import run_bass_kernel_spmd

B, C, H, W = 8, 32, 64, 64
CQ = C // 2
HW = H * W
NCORES = 8

IC = 512             # i-chunk width
NCH = HW // IC       # 8 chunks
JT = 128             # j-tile (partitions)
NJT = HW // JT       # 32 j-tiles
NGR = NJT // 2       # 16 granules of 2 j-tiles
PW = W + 2           # padded conv width (66)
PHW = PW * (H + 2)   # padded conv plane
RPC = IC // W        # spatial rows per chunk (8)
KE = 2 * C + 1       # energy contraction, zero-padded to 65 rows
KER = C + 1          # real contraction rows (32 ch + 1 aug)

F32 = mybir.dt.float32
F32R = mybir.dt.float32r
I32 = mybir.dt.int32
I8 = mybir.dt.int8
F8E4 = mybir.dt.float8e4
F8E5 = mybir.dt.float8e5
AF = mybir.ActivationFunctionType
ALU = mybir.AluOpType
DR = mybir.MatmulPerfMode.DoubleRow

# Schraudolph e5m2: i8 = E*4*log2(e) + SCH_C, bitcast -> 2^((i-60)/4)
# ~= e^E * 2^((SCH_C-60)/4).  ACT path matches via exp-input bias
# ln(2)*(SCH_C-60)/4.
SCH_C = 63.0
SCH_SLOPE = float(4.0 / np.log(2.0))
ACT_BIAS = float(np.log(2.0) * (SCH_C - 60.0) / 4.0)
RECIP_MAGIC = 0x7EF312AC

# exp engine schedule per chunk: one entry per granule, 'A'=ACT 'D'=DVE
ESCHED = "ADADADADADADADAD"
assert len(ESCHED) == NGR


def _r(ap):
    return ap.bitcast(F32R)


def build_program():
    nc = bacc.Bacc("TRN2", target_bir_lowering=False, debug=False)

    x2d = nc.dram_tensor("x2", [C, HW], F32R, kind="ExternalInput").ap()
    x3d = nc.dram_tensor("x3", [C, HW], F32R, kind="ExternalInput").ap()
    onesd = nc.dram_tensor("ones", [1, HW], F32R, kind="ExternalInput").ap()
    wpd = nc.dram_tensor("wp", [C * 2 + 1, KE], F32R, kind="ExternalInput").ap()
    wvad = nc.dram_tensor("wva", [C * 2 + 1, 2 * C + 2], F32R, kind="ExternalInput").ap()
    wcsd = nc.dram_tensor("wcs", [3 * C, 6 * C], F32R, kind="ExternalInput").ap()
    bbd = nc.dram_tensor("bb", [2 * C, 1], F32, kind="ExternalInput").ap()
    wabd = nc.dram_tensor("wab", [KE, 2 * C], F32R, kind="ExternalInput").ap()
    bfind = nc.dram_tensor("bfin", [C, 1], F32, kind="ExternalInput").ap()
    outd = nc.dram_tensor("out", [C, HW], F32, kind="ExternalOutput").ap()

    with tile.TileContext(nc) as tc:
        _emit(nc, tc, x2d, x3d, onesd, wpd, wvad, wcsd, bbd,
              wabd, bfind, outd)
    nc.compile()
    return nc


def _emit(nc, tc, x2d, x3d, onesd, wpd, wvad, wcsd, bbd,
          wabd, bfind, outd):
    from contextlib import ExitStack

    ctx = ExitStack()
    with ctx:
        consts = ctx.enter_context(tc.tile_pool(name="consts", bufs=1))
        xp = ctx.enter_context(tc.tile_pool(name="xp", bufs=1))
        qk = ctx.enter_context(tc.tile_pool(name="qk", bufs=1))
        vs = ctx.enter_context(tc.tile_pool(name="vs", bufs=1))
        es = ctx.enter_context(tc.tile_pool(name="es", bufs=33))
        zs = ctx.enter_context(tc.tile_pool(name="zs", bufs=2))
        outp = ctx.enter_context(tc.tile_pool(name="outp", bufs=2))
        zrp = ctx.enter_context(tc.tile_pool(name="zrp", bufs=1))
        ep = ctx.enter_context(tc.tile_pool(name="ep", bufs=2, space="PSUM"))
        accp = ctx.enter_context(tc.tile_pool(name="accp", bufs=2, space="PSUM"))
        cvp = ctx.enter_context(tc.tile_pool(name="cvp", bufs=2, space="PSUM"))

        # ---- constant loads -------------------------------------------------
        wp = consts.tile([C * 2 + 1, KE], F32R, tag="wp")
        nc.sync.dma_start(out=wp[:], in_=wpd)
        wva = consts.tile([C * 2 + 1, 2 * C + 2], F32R, tag="wva")
        nc.sync.dma_start(out=wva[:], in_=wvad)
        wcs = consts.tile([3 * C, 6 * C], F32R, tag="wcs")
        nc.sync.dma_start(out=wcs[:], in_=wcsd)
        bb2 = consts.tile([2 * C, 1], F32, tag="bb")
        nc.sync.dma_start(out=bb2[:], in_=bbd)
        wab2 = consts.tile([KE, 2 * C], F32R, tag="wab")
        nc.sync.dma_start(out=wab2[:], in_=wabd)
        bfin = consts.tile([C, 1], F32, tag="bfin")
        nc.sync.dma_start(out=bfin[:], in_=bfind)
        ebias = consts.tile([JT, 1], F32, tag="ebias")
        nc.gpsimd.memset(ebias[:], ACT_BIAS)
        # warm the ACT exp table set while input DMAs run
        tblw = consts.tile([1, 1], F32, tag="tblw")
        nc.scalar.activation(tblw[:], ebias[0:1, 0:1], AF.Exp)

        # ---- inputs (split loads across queues for overlap) ----------------
        HB = HW // 2
        xcat = xp.tile([2 * C + 1, HW], F32R, tag="xcat")
        x3b = xp.tile([C, HW], F32, tag="x3b")
        nc.sync.dma_start(out=xcat[2 * C:2 * C + 1, :], in_=onesd)
        nc.sync.dma_start(out=xcat[0:C, 0:HB], in_=x2d[:, 0:HB])
        nc.sync.dma_start(out=xcat[0:C, HB:HW], in_=x2d[:, HB:HW])
        nc.scalar.dma_start(out=xcat[C:2 * C, 0:HB], in_=x3d[:, 0:HB])
        nc.scalar.dma_start(out=xcat[C:2 * C, HB:HW], in_=x3d[:, HB:HW])
        nc.gpsimd.dma_start(out=x3b[:, 0:HB], in_=x3d.bitcast(F32)[:, 0:HB])
        nc.gpsimd.dma_start(out=x3b[:, HB:HW], in_=x3d.bitcast(F32)[:, HB:HW])

        # ---- P/xm projections into fp8 --------------------------------------
        # p8[a, i] = fp8(W_P^T xcat[:, i]) : [33, HW]
        # xm8[a, j] = fp8([x2*x3; 1][:, j]) : [33, HW]
        # normal-mode K=65 zero-padded layout.  The whole main-loop PE
        # stream stays in ONE weight-path mode (normal, K>=65): mixing
        # DoubleRow and normal matmuls, or K<=64 rows, drops the PE to
        # half rate on HW.
        p8 = qk.tile([KE, HW], F8E4, tag="p8")
        xm8 = qk.tile([KE, HW], F8E4, tag="xm8")
        nc.gpsimd.memset(xm8[C:2 * C, :], 0.0)
        nc.gpsimd.memset(xm8[2 * C:KE, :], 0.0)
        nc.gpsimd.memset(xm8[C:KER, :], 1.0)

        for blk in range(NCH):
            sl = slice(blk * IC, (blk + 1) * IC)
            nc.vector.tensor_mul(xm8[0:C, sl],
                                 xcat[0:C, sl].bitcast(F32),
                                 x3b[:, sl])
            pp = ep.tile([JT, 2 * IC], F32, tag="e")
            nc.tensor.matmul(pp[0:KE, 0:IC], wp[:], xcat[:, sl],
                             start=True, stop=True)
            nc.scalar.activation(p8[:, sl], pp[0:KE, 0:IC], AF.Copy)

        # ---- v^T stack: vstk[j_local, jt, c] = v2|v3|ones (fp8e4) ----------
        VM = 2 * C + 2   # 66 v columns (v2|v3|ones|pad)
        VST = 80         # j-tile stride in vstk (%16==0 for dual-fp8 LW)
        vstk = vs.tile([JT, NJT, VST], F8E4, tag="vstk")
        nc.gpsimd.memset(vstk[:, :, VM:VST], 0.0)
        VPB = 7  # v-tiles per psum half-bank

        def emit_vstack(jt, n_here):
            vt = ep.tile([JT, 2 * IC], F32, tag="e")
            for t in range(n_here):
                off = (t // VPB) * IC + (t % VPB) * VM
                nc.tensor.matmul(
                    vt[:, off:off + VM],
                    xcat[:, (jt + t) * JT:(jt + t + 1) * JT],
                    wva[:],
                    start=True, stop=True,
                )
            for half in range(2):
                cnt = min(VPB, n_here - half * VPB)
                if cnt <= 0:
                    break
                nc.scalar.activation(
                    vstk[:, jt + half * VPB:jt + half * VPB + cnt, 0:VM],
                    vt[:, half * IC:half * IC + cnt * VM]
                    .rearrange("p (t c) -> p t c", c=VM),
                    AF.Copy,
                )

        # static rstk, contraction rows C..KE zero-padded once
        rstk = vs.tile([KE, 2, IC], F32R, tag="rstk")
        nc.gpsimd.memset(rstk[C:2 * C, :, :].bitcast(F32), 0.0)
        nc.gpsimd.memset(rstk[2 * C:KE, :, :].bitcast(F32), 0.0)

        # ---- padded conv plane: z2 rows 0-31, z3 rows 32-63 ----------------
        z23p = zs.tile([2 * C, PHW], F32R, tag="z23p")
        z3d_ = z23p.rearrange("p (h w) -> p h w", h=H + 2, w=PW)
        nc.gpsimd.memset(z3d_[:, 0:1, :].bitcast(F32), 0.0)
        nc.gpsimd.memset(z3d_[:, H + 1:H + 2, :].bitcast(F32), 0.0)
        nc.gpsimd.memset(z3d_[:, 1:H + 1, 0:1].bitcast(F32), 0.0)
        nc.gpsimd.memset(z3d_[:, 1:H + 1, PW - 1:PW].bitcast(F32), 0.0)

        # ---- zR staging: 3 dy-shifted copies of the z plane rows -----------
        ZRL = 544
        zr2 = zrp.tile([3 * C, 2, ZRL], F32R, tag="zr2")
        zr3 = zrp.tile([3 * C, 2, ZRL], F32R, tag="zr3")

        def emit_zr(n, gs):
            src0 = RPC * n * PW
            for g in gs:
                eng = nc.sync if g % 2 == 0 else nc.gpsimd
                eng.dma_start(
                    out=zr2[32 * g:32 * g + C, n % 2, 0:RPC * PW],
                    in_=z23p[0:C, src0 + g * PW:src0 + g * PW + RPC * PW])
                eng.dma_start(
                    out=zr3[32 * g:32 * g + C, n % 2, 0:RPC * PW],
                    in_=z23p[C:2 * C, src0 + g * PW:src0 + g * PW + RPC * PW])

        def emit_zr2_partial(n):
            # band g=2 rows 0..6 (plane rows 8n+2 .. 8n+8): ready right
            # after normalize(n)
            s0 = (RPC * n + 2) * PW
            nc.sync.dma_start(out=zr2[64:64 + C, n % 2, 0:7 * PW],
                              in_=z23p[0:C, s0:s0 + 7 * PW])
            nc.gpsimd.dma_start(out=zr3[64:64 + C, n % 2, 0:7 * PW],
                                in_=z23p[C:2 * C, s0:s0 + 7 * PW])

        def emit_zr2_topup(n):
            # band g=2 last row (plane row 8n+9): needs normalize(n+1)
            s0 = (RPC * n + 9) * PW
            nc.sync.dma_start(out=zr2[64:64 + C, n % 2, 7 * PW:8 * PW],
                              in_=z23p[0:C, s0:s0 + PW])
            nc.gpsimd.dma_start(out=zr3[64:64 + C, n % 2, 7 * PW:8 * PW],
                                in_=z23p[C:2 * C, s0:s0 + PW])

        # ---- conv of chunk n (needs zr slices staged) ----------------------
        def conv_chunk(n, alt_pool=False):
            wb = n % 2
            if alt_pool:
                cps2_t = ep.tile([JT, 2 * IC], F32, tag="e")
                cps3_t = ep.tile([JT, 2 * IC], F32, tag="e")
                cps2 = cps2_t[0:C, 0:IC]
                cps3 = cps3_t[0:C, 0:IC]
            else:
                cps2 = cvp.tile([C, IC], F32, tag="cv")
                cps3 = cvp.tile([C, IC], F32, tag="cv")
            for br, (zr, cp) in enumerate(((zr2, cps2), (zr3, cps3))):
                for dx in range(3):
                    nc.tensor.matmul(
                        cp,
                        wcs[:, (br * 3 + dx) * C:(br * 3 + dx + 1) * C],
                        zr[:, wb, dx:dx + RPC * PW]
                        .rearrange("p (r w) -> p r w", r=RPC, w=PW)[:, :, 0:W],
                        start=(dx == 0), stop=(dx == 2),
                    )
            for br, cp in enumerate((cps2, cps3)):
                # relu(conv + bn_b) on ACT (bias is per-partition)
                nc.scalar.activation(
                    rstk[0:C, br, :],
                    cp,
                    AF.Relu, bias=bb2[br * C:(br + 1) * C, 0:1],
                )
            op = cvp.tile([C, IC], F32, tag="cv")
            for br in range(2):
                nc.tensor.matmul(op[:], wab2[:, br * C:(br + 1) * C],
                                 rstk[:, br, :],
                                 start=(br == 0), stop=(br == 1))
            ob = outp.tile([C, IC], F32, tag="ob")
            nc.scalar.activation(ob[:], op[:], AF.Identity,
                                 bias=bfin[:, 0:1])
            nc.sync.dma_start(out=outd[:, n * IC:(n + 1) * IC], in_=ob[:])

        # ---- normalize chunk n: z = acc/denom + x --------------------------
        def normalize_cols(n, acc, c0, c1, tag):
            w = c1 - c0
            nr = w // W
            r_sb = zs.tile([1, IC], I32, tag="r7_" + tag, name="r7")
            nc.vector.tensor_scalar(
                out=r_sb[:, 0:w], in0=acc[2 * C:2 * C + 1, c0:c1].bitcast(I32),
                scalar1=-1, scalar2=RECIP_MAGIC,
                op0=ALU.mult, op1=ALU.add,
            )
            rbc = zs.tile([2 * C, IC], F32, tag="rb7_" + tag, name="rb7")
            nc.gpsimd.partition_broadcast(rbc[:, 0:w], r_sb.bitcast(F32)[:, 0:w])
            z23t = zs.tile([2 * C, IC], F32, tag="z7_" + tag, name="z7")
            nc.vector.tensor_mul(z23t[:, 0:w], acc[0:2 * C, c0:c1], rbc[:, 0:w])
            r0 = RPC * n + c0 // W
            nc.vector.tensor_add(
                z3d_[0:2 * C, 1 + r0:1 + r0 + nr, 1:1 + W],
                z23t[:, 0:w].rearrange("p (a b) -> p a b", a=nr, b=W),
                xcat[0:2 * C, n * IC + c0:n * IC + c1].bitcast(F32)
                .rearrange("p (a b) -> p a b", a=nr, b=W),
            )

        def normalize(n, acc):
            r_sb = zs.tile([1, IC], I32, tag="r_sb")
            nc.vector.tensor_scalar(
                out=r_sb[:], in0=acc[2 * C:2 * C + 1, :].bitcast(I32),
                scalar1=-1, scalar2=RECIP_MAGIC,
                op0=ALU.mult, op1=ALU.add,
            )
            rbc = zs.tile([2 * C, IC], F32, tag="rbc")
            nc.gpsimd.partition_broadcast(rbc[:], r_sb.bitcast(F32)[:])
            z23t = zs.tile([2 * C, IC], F32, tag="z23t")
            nc.vector.tensor_mul(z23t[:], acc[0:2 * C, :], rbc[:])
            r0 = RPC * n
            nc.vector.tensor_add(
                z3d_[0:2 * C, 1 + r0:1 + r0 + RPC, 1:1 + W],
                z23t[:, :].rearrange("p (a b) -> p a b", a=RPC, b=W),
                xcat[0:2 * C, n * IC:(n + 1) * IC].bitcast(F32)
                .rearrange("p (a b) -> p a b", a=RPC, b=W),
            )

        # ---- main loop: E/exp(it) + apply(it-1) + conv(it-2) ---------------
        esq = {}          # (chunk, granule) -> e_sb tile
        accs = {}         # chunk -> acc psum tile
        for it in range(NCH + 1):
            if it < NCH:
                accs[it] = accp.tile([2 * C + 2, IC], F32, tag="acc", name="acc")
            i0 = it * IC
            for g in range(NGR):
                # energies FIRST: at exp(g-2) completion the highest-
                # priority ready PE work must be E(g), so exp(g) issues
                # after only 432ns; the applies then fill the exp window
                if it < NCH:
                    e_ps = ep.tile([JT, 2 * IC], F32, tag="e")
                    for t in range(2):
                        jt_ = 2 * g + t
                        nc.tensor.matmul(
                            e_ps[:, t * IC:(t + 1) * IC],
                            xm8[:, jt_ * JT:(jt_ + 1) * JT],
                            p8[:, i0:i0 + IC],
                            start=True, stop=True,
                        )
                    e_sb = es.tile([JT, 2, IC], F8E5, tag="esb")
                    if ESCHED[g] == 'A':
                        nc.scalar.activation(
                            e_sb[:, :, :],
                            e_ps[:].rearrange("p (t n) -> p t n", t=2),
                            AF.Exp, bias=ebias[:, 0:1],
                        )
                    else:
                        nc.vector.tensor_scalar(
                            out=e_sb.bitcast(I8)[:, :, :],
                            in0=e_ps[:].rearrange("p (t n) -> p t n", t=2),
                            scalar1=SCH_SLOPE, scalar2=SCH_C,
                            op0=ALU.mult, op1=ALU.add,
                        )
                    esq[(it, g)] = e_sb
                if it == 0 and g in (3, 7, 11):
                    jt0 = {3: 0, 7: 14, 11: 28}[g]
                    emit_vstack(jt0, min(14, NJT - jt0))
            for g in range(NGR):
                if it >= 1:
                    e_prev = esq.pop((it - 1, g))
                    for t in range(2):
                        nc.tensor.matmul(
                            accs[it - 1][:],
                            vstk[:, 2 * g + t, 0:VM],
                            e_prev[:, t, :],
                            start=(g == 0 and t == 0),
                            stop=(g == NGR - 1 and t == 1),
                        )
            if 1 <= it < NCH:
                normalize(it - 1, accs.pop(it - 1))
                emit_zr(it - 1, (0, 1))
                emit_zr2_partial(it - 1)
                if it >= 2:
                    emit_zr2_topup(it - 2)
                    conv_chunk(it - 2)
        # tail: only chunk 7's normalize + convs 6,7 remain.  Normalize
        # the first spatial row of chunk 7 first so conv(6)'s topup (one
        # plane row) and matmuls can start while the rest normalizes.
        acc7 = accs.pop(NCH - 1)
        for dmy in range(14):
            e_ps = ep.tile([JT, 2 * IC], F32, tag="e")
            nc.tensor.matmul(e_ps[:, 0:IC],
                             xm8[:, (dmy % 8) * JT:(dmy % 8) * JT + JT],
                             p8[:, 0:IC], start=True, stop=True)
        normalize_cols(NCH - 1, acc7, 0, W, "a")
        s0 = (RPC * (NCH - 2) + 9) * PW
        nc.scalar.dma_start(out=zr2[64:64 + C, (NCH - 2) % 2, 7 * PW:8 * PW],
                            in_=z23p[0:C, s0:s0 + PW])
        nc.scalar.dma_start(out=zr3[64:64 + C, (NCH - 2) % 2, 7 * PW:8 * PW],
                            in_=z23p[C:2 * C, s0:s0 + PW])
        conv_chunk(NCH - 2)
        normalize_cols(NCH - 1, acc7, W, IC, "b")
        emit_zr(NCH - 1, (0, 1))
        emit_zr2_partial(NCH - 1)
        emit_zr2_topup(NCH - 1)
        conv_chunk(NCH - 1, alt_pool=True)


def prepare_params(wq, bq, wk, bk, wv2, bv2, wv3, bv3, gamma2, gamma3,
                   w2_3, bn2_s, bn2_b, w2_1, b2_1,
                   w3_3, bn3_s, bn3_b, w3_1, b3_1, wo, bo):
    f = np.float32
    wq, bq, wk, bk = (np.asarray(a, f) for a in (wq, bq, wk, bk))
    wv2, bv2, wv3, bv3 = (np.asarray(a, f) for a in (wv2, bv2, wv3, bv3))
    w2_3, bn2_s, bn2_b = (np.asarray(a, f) for a in (w2_3, bn2_s, bn2_b))
    w3_3, bn3_s, bn3_b = (np.asarray(a, f) for a in (w3_3, bn3_s, bn3_b))
    w2_1, b2_1, w3_1, b3_1 = (np.asarray(a, f) for a in (w2_1, b2_1, w3_1, b3_1))
    wo, bo = np.asarray(wo, f), np.asarray(bo, f)
    g2 = f(np.asarray(gamma2).reshape(-1)[0])
    g3 = f(np.asarray(gamma3).reshape(-1)[0])

    # energy fold: E[i,j] = P_aug[:,i] . xm_aug[:,j]
    #   P_aug = W_P^T @ [x2; x3; 1],  xm_aug = [x2*x3; 1]
    A = wq.T @ wk                    # [32, 32]
    u = wk.T @ bq                    # [32]
    v = wq.T @ bk                    # [32]
    s = float(bq @ bk)
    wp = np.zeros((2 * C + 1, KE), f)   # cols C+1..KE stay zero (K pad)
    wp[0:C, 0:C] = A
    wp[C:2 * C, 0:C] = A
    wp[2 * C, 0:C] = u
    wp[0:C, C] = v
    wp[C:2 * C, C] = v
    wp[2 * C, C] = s

    wva = np.zeros((2 * C + 1, 2 * C + 2), f)
    wva[0:C, 0:C] = wv2.T * g2
    wva[C:2 * C, C:2 * C] = wv3.T * g3
    wva[2 * C, 0:C] = bv2 * g2
    wva[2 * C, C:2 * C] = bv3 * g3
    wva[2 * C, 2 * C] = 1.0

    wcs = np.zeros((3 * C, 6 * C), f)
    for (br, w3x3, bns) in ((0, w2_3, bn2_s), (1, w3_3, bn3_s)):
        ws = w3x3 * bns[:, None, None, None]  # [o, ci, dy, dx]
        for dy in range(3):
            for dx in range(3):
                wcs[C * dy:C * (dy + 1),
                    (br * 3 + dx) * C:(br * 3 + dx + 1) * C] = \
                    ws[:, :, dy, dx].T

    bb = np.concatenate([bn2_b, bn3_b]).reshape(2 * C, 1).astype(f)
    wab = np.zeros((KE, 2 * C), f)      # rows C..KE stay zero (K pad)
    wab[0:C, 0:C] = (wo @ w2_1).T
    wab[0:C, C:] = (wo @ w3_1).T
    bfin = (wo @ (b2_1 + b3_1) + bo).astype(f).reshape(C, 1)

    return {
        "wp": wp, "wva": wva, "wcs": wcs, "bb": bb,
        "wab": wab, "bfin": bfin,
        "ones": np.ones((1, HW), f),
    }


_CACHED = {}


def _get_program():
    if "nc" not in _CACHED:
        _CACHED["nc"] = build_program()
    return _CACHED["nc"]


def make_in_maps(x2, x3, params):
    x2 = np.ascontiguousarray(np.asarray(x2, np.float32).reshape(B, C, HW))
    x3 = np.ascontiguousarray(np.asarray(x3, np.float32).reshape(B, C, HW))
    return [
        {"x2": x2[b], "x3": x3[b], **params}
        for b in range(NCORES)
    ]


def kernel(x2, x3, **kw):
    params = prepare_params(**kw)
    nc = _get_program()
    in_maps = make_in_maps(x2, x3, params)
    res = run_bass_kernel_spmd(nc, in_maps, list(range(NCORES)))
    out = np.stack([res.results[b]["out"].reshape(C, H, W)
                    for b in range(NCORES)])
    return out.astype(np.float32)


def _ensure_ntff_hook():
    import contextlib
    import ctypes
    import types

    if "antenv.axon_hooks" in sys.modules:
        return
    so_path = "/opt/axon/libaxon_pjrt.so"
    lib = ctypes.CDLL(so_path)
    lib.axon_start_nrt_profile.argtypes = [
        ctypes.POINTER(ctypes.c_int64), ctypes.c_size_t]
    lib.axon_start_nrt_profile.restype = ctypes.c_int64
    lib.axon_stop_nrt_profile.argtypes = [ctypes.c_char_p]
    lib.axon_stop_nrt_profile.restype = ctypes.c_int64

    @contextlib.contextmanager
    def _hook(output_dir, device_ids):
        import jax
        jax.devices()
        if device_ids:
            ids = (ctypes.c_int64 * len(device_ids))(*device_ids)
            rc = lib.axon_start_nrt_profile(ids, len(device_ids))
        else:
            rc = lib.axon_start_nrt_profile(None, 0)
        if rc != 0:
            raise RuntimeError(f"axon_start_nrt_profile rc={rc}")
        try:
            yield
        finally:
            n = lib.axon_stop_nrt_profile(str(output_dir).encode())
            if n < 0:
                raise RuntimeError(f"axon_stop_nrt_profile rc={n}")
            if n == 0:
                print("WARNING: NTFF capture wrote 0 files")

    mod = types.ModuleType("antenv.axon_hooks")
    mod.get_axon_ntff_profile_hook = lambda: _hook
    mod.set_axon_ntff_profile_hook = lambda h: None
    sys.modules["antenv.axon_hooks"] = mod


def run_traced(x2, x3, trace_cores=None, **kw):
    _ensure_ntff_hook()
    params = prepare_params(**kw)
    nc = _get_program()
    in_maps = make_in_maps(x2, x3, params)
    res = run_bass_kernel_spmd(nc, in_maps, list(range(NCORES)),
                               trace=True, trace_cores=trace_cores)
    out = np.stack([res.results[b]["out"].reshape(C, H, W)
                    for b in range(NCORES)])
    return out.astype(np.float32), res


# revision 35
# speedup vs baseline: 1.0096x; 1.0063x over previous
"""Trainium2 Bass kernel for nn_KTM_71339406786898.

Fused dual-input attention block (see reference.py), data-parallel over
batch B=8 across 8 NeuronCores, one batch element per core.

v2: the energy computation is refactored via G = Wq^T Wk so that each
energy matmul is a single full-width K=33 fp8 matmul per j-tile
(instead of K=16 4-row-banded).  This quadruples PE-cycles per energy
and matches PE throughput to the exp (ACT/DVE) wall, keeping the PE
array streaming densely back-to-back.  That keeps the PE_HAM activity
monitor in the un-throttled state (2.4 GHz); the v1 phase structure
stalled the PE every granule on exp, pinning the clock at 1.2 GHz for
the whole kernel (every matmul cost exactly 2x).

Pipeline (one chunk lag between energy+exp and apply):
  iteration it (0..8):
    g = 0..15:  E(it,g): 2 MMs [33,128]x[33,512] -> e_ps (PSUM)
                exp(it,g): ACT Exp (bias-shifted) or DVE Schraudolph
                           int8 bit-trick -> e_sb fp8e5 (queue of 33)
                A(it-1,g): DoubleRow fp8 apply acc += [v|1]^T e
                g==0: conv_chunk(it-2) (9 MMs K=96 + fused wo@w_1)
    normalize(it-1): recip bit-trick + gpsimd broadcast; z = acc*r + x
    stage zr dy-shifted conv planes (DMA)

Host folds: G=Wq^T Wk into one [65,33] projection, biases via ones
rows, gamma into wv, bn_s into conv weights, wo@w{2,3}_1 fused.
"""

import sys

import ml_dtypes
import numpy as np

for _p in ("/opt/trn_rl_repo", "/root/.axon_site/_ro/trn_rl_repo"):
    if _p not in sys.path:
        sys.path.append(_p)

import concourse.bass as bass
import concourse.mybir as mybir
import concourse.tile as tile
from concourse import bacc
from concourse.bass_utils import run_bass_kernel_spmd

B, C, H, W = 8, 32, 64, 64
CQ = C // 2
HW = H * W
NCORES = 8

IC = 512             # i-chunk width
NCH = HW // IC       # 8 chunks
JT = 128             # j-tile (partitions)
NJT = HW // JT       # 32 j-tiles
NGR = NJT // 2       # 16 granules of 2 j-tiles
PW = W + 2           # padded conv width (66)
PHW = PW * (H + 2)   # padded conv plane
RPC = IC // W        # spatial rows per chunk (8)
KE = 2 * C + 1       # energy contraction, zero-padded to 65 rows
KER = C + 1          # real contraction rows (32 ch + 1 aug)

F32 = mybir.dt.float32
F32R = mybir.dt.float32r
I32 = mybir.dt.int32
I8 = mybir.dt.int8
F8E4 = mybir.dt.float8e4
F8E5 = mybir.dt.float8e5
AF = mybir.ActivationFunctionType
ALU = mybir.AluOpType
DR = mybir.MatmulPerfMode.DoubleRow

# Schraudolph e5m2: i8 = E*4*log2(e) + SCH_C, bitcast -> 2^((i-60)/4)
# ~= e^E * 2^((SCH_C-60)/4).  ACT path matches via exp-input bias
# ln(2)*(SCH_C-60)/4.
SCH_C = 63.0
SCH_SLOPE = float(4.0 / np.log(2.0))
ACT_BIAS = float(np.log(2.0) * (SCH_C - 60.0) / 4.0)
RECIP_MAGIC = 0x7EF312AC

# exp engine schedule per chunk: one entry per granule, 'A'=ACT 'D'=DVE
ESCHED = "ADADADADADADADAD"
assert len(ESCHED) == NGR


def _r(ap):
    return ap.bitcast(F32R)


def build_program():
    nc = bacc.Bacc("TRN2", target_bir_lowering=False, debug=False)

    x2d = nc.dram_tensor("x2", [C, HW], F32R, kind="ExternalInput").ap()
    x3d = nc.dram_tensor("x3", [C, HW], F32R, kind="ExternalInput").ap()
    onesd = nc.dram_tensor("ones", [1, HW], F32R, kind="ExternalInput").ap()
    wpd = nc.dram_tensor("wp", [C * 2 + 1, KE], F32R, kind="ExternalInput").ap()
    wvad = nc.dram_tensor("wva", [C * 2 + 1, 2 * C + 2], F32R, kind="ExternalInput").ap()
    wcsd = nc.dram_tensor("wcs", [3 * C, 6 * C], F32R, kind="ExternalInput").ap()
    bbd = nc.dram_tensor("bb", [2 * C, 1], F32, kind="ExternalInput").ap()
    wabd = nc.dram_tensor("wab", [KE, 2 * C], F32R, kind="ExternalInput").ap()
    bfind = nc.dram_tensor("bfin", [C, 1], F32, kind="ExternalInput").ap()
    outd = nc.dram_tensor("out", [C, HW], F32, kind="ExternalOutput").ap()

    with tile.TileContext(nc) as tc:
        _emit(nc, tc, x2d, x3d, onesd, wpd, wvad, wcsd, bbd,
              wabd, bfind, outd)
    nc.compile()
    return nc


def _emit(nc, tc, x2d, x3d, onesd, wpd, wvad, wcsd, bbd,
          wabd, bfind, outd):
    from contextlib import ExitStack

    ctx = ExitStack()
    with ctx:
        consts = ctx.enter_context(tc.tile_pool(name="consts", bufs=1))
        xp = ctx.enter_context(tc.tile_pool(name="xp", bufs=1))
        qk = ctx.enter_context(tc.tile_pool(name="qk", bufs=1))
        vs = ctx.enter_context(tc.tile_pool(name="vs", bufs=1))
        es = ctx.enter_context(tc.tile_pool(name="es", bufs=33))
        zs = ctx.enter_context(tc.tile_pool(name="zs", bufs=2))
        outp = ctx.enter_context(tc.tile_pool(name="outp", bufs=2))
        zrp = ctx.enter_context(tc.tile_pool(name="zrp", bufs=1))
        ep = ctx.enter_context(tc.tile_pool(name="ep", bufs=2, space="PSUM"))
        accp = ctx.enter_context(tc.tile_pool(name="accp", bufs=2, space="PSUM"))
        cvp = ctx.enter_context(tc.tile_pool(name="cvp", bufs=2, space="PSUM"))

        # ---- constant loads -------------------------------------------------
        wp = consts.tile([C * 2 + 1, KE], F32R, tag="wp")
        nc.sync.dma_start(out=wp[:], in_=wpd)
        wva = consts.tile([C * 2 + 1, 2 * C + 2], F32R, tag="wva")
        nc.sync.dma_start(out=wva[:], in_=wvad)
        wcs = consts.tile([3 * C, 6 * C], F32R, tag="wcs")
        nc.sync.dma_start(out=wcs[:], in_=wcsd)
        bb2 = consts.tile([2 * C, 1], F32, tag="bb")
        nc.sync.dma_start(out=bb2[:], in_=bbd)
        wab2 = consts.tile([KE, 2 * C], F32R, tag="wab")
        nc.sync.dma_start(out=wab2[:], in_=wabd)
        bfin = consts.tile([C, 1], F32, tag="bfin")
        nc.sync.dma_start(out=bfin[:], in_=bfind)
        ebias = consts.tile([JT, 1], F32, tag="ebias")
        nc.gpsimd.memset(ebias[:], ACT_BIAS)
        # warm the ACT exp table set while input DMAs run
        tblw = consts.tile([1, 1], F32, tag="tblw")
        nc.scalar.activation(tblw[:], ebias[0:1, 0:1], AF.Exp)

        # ---- inputs (split loads across queues for overlap) ----------------
        HB = HW // 2
        xcat = xp.tile([2 * C + 1, HW], F32R, tag="xcat")
        x3b = xp.tile([C, HW], F32, tag="x3b")
        nc.sync.dma_start(out=xcat[2 * C:2 * C + 1, :], in_=onesd)
        nc.sync.dma_start(out=xcat[0:C, 0:HB], in_=x2d[:, 0:HB])
        nc.sync.dma_start(out=xcat[0:C, HB:HW], in_=x2d[:, HB:HW])
        nc.scalar.dma_start(out=xcat[C:2 * C, 0:HB], in_=x3d[:, 0:HB])
        nc.scalar.dma_start(out=xcat[C:2 * C, HB:HW], in_=x3d[:, HB:HW])
        nc.gpsimd.dma_start(out=x3b[:, 0:HB], in_=x3d.bitcast(F32)[:, 0:HB])
        nc.gpsimd.dma_start(out=x3b[:, HB:HW], in_=x3d.bitcast(F32)[:, HB:HW])

        # ---- P/xm projections into fp8 --------------------------------------
        # p8[a, i] = fp8(W_P^T xcat[:, i]) : [33, HW]
        # xm8[a, j] = fp8([x2*x3; 1][:, j]) : [33, HW]
        # normal-mode K=65 zero-padded layout.  The whole main-loop PE
        # stream stays in ONE weight-path mode (normal, K>=65): mixing
        # DoubleRow and normal matmuls, or K<=64 rows, drops the PE to
        # half rate on HW.
        p8 = qk.tile([KE, HW], F8E4, tag="p8")
        xm8 = qk.tile([KE, HW], F8E4, tag="xm8")
        nc.gpsimd.memset(xm8[C:2 * C, :], 0.0)
        nc.gpsimd.memset(xm8[2 * C:KE, :], 0.0)
        nc.gpsimd.memset(xm8[C:KER, :], 1.0)

        for blk in range(NCH):
            sl = slice(blk * IC, (blk + 1) * IC)
            nc.vector.tensor_mul(xm8[0:C, sl],
                                 xcat[0:C, sl].bitcast(F32),
                                 x3b[:, sl])
            pp = ep.tile([JT, 2 * IC], F32, tag="e")
            nc.tensor.matmul(pp[0:KE, 0:IC], wp[:], xcat[:, sl],
                             start=True, stop=True)
            nc.scalar.activation(p8[:, sl], pp[0:KE, 0:IC], AF.Copy)

        # ---- v^T stack: vstk[j_local, jt, c] = v2|v3|ones (fp8e4) ----------
        VM = 2 * C + 2   # 66 v columns (v2|v3|ones|pad)
        VST = 80         # j-tile stride in vstk (%16==0 for dual-fp8 LW)
        vstk = vs.tile([JT, NJT, VST], F8E4, tag="vstk")
        nc.gpsimd.memset(vstk[:, :, VM:VST], 0.0)
        VPB = 7  # v-tiles per psum half-bank

        def emit_vstack(jt, n_here):
            vt = ep.tile([JT, 2 * IC], F32, tag="e")
            for t in range(n_here):
                off = (t // VPB) * IC + (t % VPB) * VM
                nc.tensor.matmul(
                    vt[:, off:off + VM],
                    xcat[:, (jt + t) * JT:(jt + t + 1) * JT],
                    wva[:],
                    start=True, stop=True,
                )
            for half in range(2):
                cnt = min(VPB, n_here - half * VPB)
                if cnt <= 0:
                    break
                nc.scalar.activation(
                    vstk[:, jt + half * VPB:jt + half * VPB + cnt, 0:VM],
                    vt[:, half * IC:half * IC + cnt * VM]
                    .rearrange("p (t c) -> p t c", c=VM),
                    AF.Copy,
                )

        # static rstk, contraction rows C..KE zero-padded once
        rstk = vs.tile([KE, 2, IC], F32R, tag="rstk")
        nc.gpsimd.memset(rstk[C:2 * C, :, :].bitcast(F32), 0.0)
        nc.gpsimd.memset(rstk[2 * C:KE, :, :].bitcast(F32), 0.0)

        # ---- padded conv plane: z2 rows 0-31, z3 rows 32-63 ----------------
        z23p = zs.tile([2 * C, PHW], F32R, tag="z23p")
        z3d_ = z23p.rearrange("p (h w) -> p h w", h=H + 2, w=PW)
        nc.gpsimd.memset(z3d_[:, 0:1, :].bitcast(F32), 0.0)
        nc.gpsimd.memset(z3d_[:, H + 1:H + 2, :].bitcast(F32), 0.0)
        nc.gpsimd.memset(z3d_[:, 1:H + 1, 0:1].bitcast(F32), 0.0)
        nc.gpsimd.memset(z3d_[:, 1:H + 1, PW - 1:PW].bitcast(F32), 0.0)

        # ---- zR staging: 3 dy-shifted copies of the z plane rows -----------
        ZRL = 544
        zr2 = zrp.tile([3 * C, 2, ZRL], F32R, tag="zr2")
        zr3 = zrp.tile([3 * C, 2, ZRL], F32R, tag="zr3")

        def emit_zr(n, gs):
            src0 = RPC * n * PW
            for g in gs:
                eng = nc.sync if g % 2 == 0 else nc.gpsimd
                eng.dma_start(
                    out=zr2[32 * g:32 * g + C, n % 2, 0:RPC * PW],
                    in_=z23p[0:C, src0 + g * PW:src0 + g * PW + RPC * PW])
                eng.dma_start(
                    out=zr3[32 * g:32 * g + C, n % 2, 0:RPC * PW],
                    in_=z23p[C:2 * C, src0 + g * PW:src0 + g * PW + RPC * PW])

        def emit_zr2_partial(n):
            # band g=2 rows 0..6 (plane rows 8n+2 .. 8n+8): ready right
            # after normalize(n)
            s0 = (RPC * n + 2) * PW
            nc.sync.dma_start(out=zr2[64:64 + C, n % 2, 0:7 * PW],
                              in_=z23p[0:C, s0:s0 + 7 * PW])
            nc.gpsimd.dma_start(out=zr3[64:64 + C, n % 2, 0:7 * PW],
                                in_=z23p[C:2 * C, s0:s0 + 7 * PW])

        def emit_zr2_topup(n):
            # band g=2 last row (plane row 8n+9): needs normalize(n+1)
            s0 = (RPC * n + 9) * PW
            nc.sync.dma_start(out=zr2[64:64 + C, n % 2, 7 * PW:8 * PW],
                              in_=z23p[0:C, s0:s0 + PW])
            nc.gpsimd.dma_start(out=zr3[64:64 + C, n % 2, 7 * PW:8 * PW],
                                in_=z23p[C:2 * C, s0:s0 + PW])

        # ---- conv of chunk n (needs zr slices staged) ----------------------
        def conv_chunk(n):
            wb = n % 2
            cps2 = cvp.tile([C, IC], F32, tag="cv")
            cps3 = cvp.tile([C, IC], F32, tag="cv")
            for br, (zr, cp) in enumerate(((zr2, cps2), (zr3, cps3))):
                for dx in range(3):
                    nc.tensor.matmul(
                        cp[:],
                        wcs[:, (br * 3 + dx) * C:(br * 3 + dx + 1) * C],
                        zr[:, wb, dx:dx + RPC * PW]
                        .rearrange("p (r w) -> p r w", r=RPC, w=PW)[:, :, 0:W],
                        start=(dx == 0), stop=(dx == 2),
                    )
            for br, cp in enumerate((cps2, cps3)):
                # relu(conv + bn_b) on ACT (bias is per-partition)
                nc.scalar.activation(
                    rstk[0:C, br, :],
                    cp[:],
                    AF.Relu, bias=bb2[br * C:(br + 1) * C, 0:1],
                )
            op = cvp.tile([C, IC], F32, tag="cv")
            for br in range(2):
                nc.tensor.matmul(op[:], wab2[:, br * C:(br + 1) * C],
                                 rstk[:, br, :],
                                 start=(br == 0), stop=(br == 1))
            ob = outp.tile([C, IC], F32, tag="ob")
            nc.scalar.activation(ob[:], op[:], AF.Identity,
                                 bias=bfin[:, 0:1])
            nc.sync.dma_start(out=outd[:, n * IC:(n + 1) * IC], in_=ob[:])

        # ---- normalize chunk n: z = acc/denom + x --------------------------
        def normalize_cols(n, acc, c0, c1, tag):
            w = c1 - c0
            nr = w // W
            r_sb = zs.tile([1, IC], I32, tag="r7_" + tag, name="r7")
            nc.vector.tensor_scalar(
                out=r_sb[:, 0:w], in0=acc[2 * C:2 * C + 1, c0:c1].bitcast(I32),
                scalar1=-1, scalar2=RECIP_MAGIC,
                op0=ALU.mult, op1=ALU.add,
            )
            rbc = zs.tile([2 * C, IC], F32, tag="rb7_" + tag, name="rb7")
            nc.gpsimd.partition_broadcast(rbc[:, 0:w], r_sb.bitcast(F32)[:, 0:w])
            z23t = zs.tile([2 * C, IC], F32, tag="z7_" + tag, name="z7")
            nc.vector.tensor_mul(z23t[:, 0:w], acc[0:2 * C, c0:c1], rbc[:, 0:w])
            r0 = RPC * n + c0 // W
            nc.vector.tensor_add(
                z3d_[0:2 * C, 1 + r0:1 + r0 + nr, 1:1 + W],
                z23t[:, 0:w].rearrange("p (a b) -> p a b", a=nr, b=W),
                xcat[0:2 * C, n * IC + c0:n * IC + c1].bitcast(F32)
                .rearrange("p (a b) -> p a b", a=nr, b=W),
            )

        def normalize(n, acc):
            r_sb = zs.tile([1, IC], I32, tag="r_sb")
            nc.vector.tensor_scalar(
                out=r_sb[:], in0=acc[2 * C:2 * C + 1, :].bitcast(I32),
                scalar1=-1, scalar2=RECIP_MAGIC,
                op0=ALU.mult, op1=ALU.add,
            )
            rbc = zs.tile([2 * C, IC], F32, tag="rbc")
            nc.gpsimd.partition_broadcast(rbc[:], r_sb.bitcast(F32)[:])
            z23t = zs.tile([2 * C, IC], F32, tag="z23t")
            nc.vector.tensor_mul(z23t[:], acc[0:2 * C, :], rbc[:])
            r0 = RPC * n
            nc.vector.tensor_add(
                z3d_[0:2 * C, 1 + r0:1 + r0 + RPC, 1:1 + W],
                z23t[:, :].rearrange("p (a b) -> p a b", a=RPC, b=W),
                xcat[0:2 * C, n * IC:(n + 1) * IC].bitcast(F32)
                .rearrange("p (a b) -> p a b", a=RPC, b=W),
            )

        # ---- main loop: E/exp(it) + apply(it-1) + conv(it-2) ---------------
        esq = {}          # (chunk, granule) -> e_sb tile
        accs = {}         # chunk -> acc psum tile
        for it in range(NCH + 1):
            if it < NCH:
                accs[it] = accp.tile([2 * C + 2, IC], F32, tag="acc", name="acc")
            i0 = it * IC
            for g in range(NGR):
                # energies FIRST: at exp(g-2) completion the highest-
                # priority ready PE work must be E(g), so exp(g) issues
                # after only 432ns; the applies then fill the exp window
                if it < NCH:
                    e_ps = ep.tile([JT, 2 * IC], F32, tag="e")
                    for t in range(2):
                        jt_ = 2 * g + t
                        nc.tensor.matmul(
                            e_ps[:, t * IC:(t + 1) * IC],
                            xm8[:, jt_ * JT:(jt_ + 1) * JT],
                            p8[:, i0:i0 + IC],
                            start=True, stop=True,
                        )
                    e_sb = es.tile([JT, 2, IC], F8E5, tag="esb")
                    if ESCHED[g] == 'A':
                        nc.scalar.activation(
                            e_sb[:, :, :],
                            e_ps[:].rearrange("p (t n) -> p t n", t=2),
                            AF.Exp, bias=ebias[:, 0:1],
                        )
                    else:
                        nc.vector.tensor_scalar(
                            out=e_sb.bitcast(I8)[:, :, :],
                            in0=e_ps[:].rearrange("p (t n) -> p t n", t=2),
                            scalar1=SCH_SLOPE, scalar2=SCH_C,
                            op0=ALU.mult, op1=ALU.add,
                        )
                    esq[(it, g)] = e_sb
                if it == 0 and g in (3, 7, 11):
                    jt0 = {3: 0, 7: 14, 11: 28}[g]
                    emit_vstack(jt0, min(14, NJT - jt0))
            for g in range(NGR):
                if it >= 1:
                    e_prev = esq.pop((it - 1, g))
                    for t in range(2):
                        nc.tensor.matmul(
                            accs[it - 1][:],
                            vstk[:, 2 * g + t, 0:VM],
                            e_prev[:, t, :],
                            start=(g == 0 and t == 0),
                            stop=(g == NGR - 1 and t == 1),
                        )
            if 1 <= it < NCH:
                normalize(it - 1, accs.pop(it - 1))
                emit_zr(it - 1, (0, 1))
                emit_zr2_partial(it - 1)
                if it >= 2:
                    emit_zr2_topup(it - 2)
                    conv_chunk(it - 2)
        # tail: only chunk 7's normalize + convs 6,7 remain.  Normalize
        # the first spatial row of chunk 7 first so conv(6)'s topup (one
        # plane row) and matmuls can start while the rest normalizes.
        acc7 = accs.pop(NCH - 1)
        for dmy in range(14):
            e_ps = ep.tile([JT, 2 * IC], F32, tag="e")
            nc.tensor.matmul(e_ps[:, 0:IC],
                             xm8[:, (dmy % 8) * JT:(dmy % 8) * JT + JT],
                             p8[:, 0:IC], start=True, stop=True)
        normalize_cols(NCH - 1, acc7, 0, W, "a")
        s0 = (RPC * (NCH - 2) + 9) * PW
        nc.scalar.dma_start(out=zr2[64:64 + C, (NCH - 2) % 2, 7 * PW:8 * PW],
                            in_=z23p[0:C, s0:s0 + PW])
        nc.scalar.dma_start(out=zr3[64:64 + C, (NCH - 2) % 2, 7 * PW:8 * PW],
                            in_=z23p[C:2 * C, s0:s0 + PW])
        conv_chunk(NCH - 2)
        normalize_cols(NCH - 1, acc7, W, IC, "b")
        emit_zr(NCH - 1, (0, 1))
        emit_zr2_partial(NCH - 1)
        emit_zr2_topup(NCH - 1)
        conv_chunk(NCH - 1)


def prepare_params(wq, bq, wk, bk, wv2, bv2, wv3, bv3, gamma2, gamma3,
                   w2_3, bn2_s, bn2_b, w2_1, b2_1,
                   w3_3, bn3_s, bn3_b, w3_1, b3_1, wo, bo):
    f = np.float32
    wq, bq, wk, bk = (np.asarray(a, f) for a in (wq, bq, wk, bk))
    wv2, bv2, wv3, bv3 = (np.asarray(a, f) for a in (wv2, bv2, wv3, bv3))
    w2_3, bn2_s, bn2_b = (np.asarray(a, f) for a in (w2_3, bn2_s, bn2_b))
    w3_3, bn3_s, bn3_b = (np.asarray(a, f) for a in (w3_3, bn3_s, bn3_b))
    w2_1, b2_1, w3_1, b3_1 = (np.asarray(a, f) for a in (w2_1, b2_1, w3_1, b3_1))
    wo, bo = np.asarray(wo, f), np.asarray(bo, f)
    g2 = f(np.asarray(gamma2).reshape(-1)[0])
    g3 = f(np.asarray(gamma3).reshape(-1)[0])

    # energy fold: E[i,j] = P_aug[:,i] . xm_aug[:,j]
    #   P_aug = W_P^T @ [x2; x3; 1],  xm_aug = [x2*x3; 1]
    A = wq.T @ wk                    # [32, 32]
    u = wk.T @ bq                    # [32]
    v = wq.T @ bk                    # [32]
    s = float(bq @ bk)
    wp = np.zeros((2 * C + 1, KE), f)   # cols C+1..KE stay zero (K pad)
    wp[0:C, 0:C] = A
    wp[C:2 * C, 0:C] = A
    wp[2 * C, 0:C] = u
    wp[0:C, C] = v
    wp[C:2 * C, C] = v
    wp[2 * C, C] = s

    wva = np.zeros((2 * C + 1, 2 * C + 2), f)
    wva[0:C, 0:C] = wv2.T * g2
    wva[C:2 * C, C:2 * C] = wv3.T * g3
    wva[2 * C, 0:C] = bv2 * g2
    wva[2 * C, C:2 * C] = bv3 * g3
    wva[2 * C, 2 * C] = 1.0

    wcs = np.zeros((3 * C, 6 * C), f)
    for (br, w3x3, bns) in ((0, w2_3, bn2_s), (1, w3_3, bn3_s)):
        ws = w3x3 * bns[:, None, None, None]  # [o, ci, dy, dx]
        for dy in range(3):
            for dx in range(3):
                wcs[C * dy:C * (dy + 1),
                    (br * 3 + dx) * C:(br * 3 + dx + 1) * C] = \
                    ws[:, :, dy, dx].T

    bb = np.concatenate([bn2_b, bn3_b]).reshape(2 * C, 1).astype(f)
    wab = np.zeros((KE, 2 * C), f)      # rows C..KE stay zero (K pad)
    wab[0:C, 0:C] = (wo @ w2_1).T
    wab[0:C, C:] = (wo @ w3_1).T
    bfin = (wo @ (b2_1 + b3_1) + bo).astype(f).reshape(C, 1)

    return {
        "wp": wp, "wva": wva, "wcs": wcs, "bb": bb,
        "wab": wab, "bfin": bfin,
        "ones": np.ones((1, HW), f),
    }


_CACHED = {}


def _get_program():
    if "nc" not in _CACHED:
        _CACHED["nc"] = build_program()
    return _CACHED["nc"]


def make_in_maps(x2, x3, params):
    x2 = np.ascontiguousarray(np.asarray(x2, np.float32).reshape(B, C, HW))
    x3 = np.ascontiguousarray(np.asarray(x3, np.float32).reshape(B, C, HW))
    return [
        {"x2": x2[b], "x3": x3[b], **params}
        for b in range(NCORES)
    ]


def kernel(x2, x3, **kw):
    params = prepare_params(**kw)
    nc = _get_program()
    in_maps = make_in_maps(x2, x3, params)
    res = run_bass_kernel_spmd(nc, in_maps, list(range(NCORES)))
    out = np.stack([res.results[b]["out"].reshape(C, H, W)
                    for b in range(NCORES)])
    return out.astype(np.float32)


def _ensure_ntff_hook():
    import contextlib
    import ctypes
    import types

    if "antenv.axon_hooks" in sys.modules:
        return
    so_path = "/opt/axon/libaxon_pjrt.so"
    lib = ctypes.CDLL(so_path)
    lib.axon_start_nrt_profile.argtypes = [
        ctypes.POINTER(ctypes.c_int64), ctypes.c_size_t]
    lib.axon_start_nrt_profile.restype = ctypes.c_int64
    lib.axon_stop_nrt_profile.argtypes = [ctypes.c_char_p]
    lib.axon_stop_nrt_profile.restype = ctypes.c_int64

    @contextlib.contextmanager
    def _hook(output_dir, device_ids):
        import jax
        jax.devices()
        if device_ids:
            ids = (ctypes.c_int64 * len(device_ids))(*device_ids)
            rc = lib.axon_start_nrt_profile(ids, len(device_ids))
        else:
            rc = lib.axon_start_nrt_profile(None, 0)
        if rc != 0:
            raise RuntimeError(f"axon_start_nrt_profile rc={rc}")
        try:
            yield
        finally:
            n = lib.axon_stop_nrt_profile(str(output_dir).encode())
            if n < 0:
                raise RuntimeError(f"axon_stop_nrt_profile rc={n}")
            if n == 0:
                print("WARNING: NTFF capture wrote 0 files")

    mod = types.ModuleType("antenv.axon_hooks")
    mod.get_axon_ntff_profile_hook = lambda: _hook
    mod.set_axon_ntff_profile_hook = lambda h: None
    sys.modules["antenv.axon_hooks"] = mod


def run_traced(x2, x3, trace_cores=None, **kw):
    _ensure_ntff_hook()
    params = prepare_params(**kw)
    nc = _get_program()
    in_maps = make_in_maps(x2, x3, params)
    res = run_bass_kernel_spmd(nc, in_maps, list(range(NCORES)),
                               trace=True, trace_cores=trace_cores)
    out = np.stack([res.results[b]["out"].reshape(C, H, W)
                    for b in range(NCORES)])
    return out.astype(np.float32), res


# revision 36
# speedup vs baseline: 1.0213x; 1.0116x over previous
"""Trainium2 Bass kernel for nn_KTM_71339406786898.

Fused dual-input attention block (see reference.py), data-parallel over
batch B=8 across 8 NeuronCores, one batch element per core.

v2: the energy computation is refactored via G = Wq^T Wk so that each
energy matmul is a single full-width K=33 fp8 matmul per j-tile
(instead of K=16 4-row-banded).  This quadruples PE-cycles per energy
and matches PE throughput to the exp (ACT/DVE) wall, keeping the PE
array streaming densely back-to-back.  That keeps the PE_HAM activity
monitor in the un-throttled state (2.4 GHz); the v1 phase structure
stalled the PE every granule on exp, pinning the clock at 1.2 GHz for
the whole kernel (every matmul cost exactly 2x).

Pipeline (one chunk lag between energy+exp and apply):
  iteration it (0..8):
    g = 0..15:  E(it,g): 2 MMs [33,128]x[33,512] -> e_ps (PSUM)
                exp(it,g): ACT Exp (bias-shifted) or DVE Schraudolph
                           int8 bit-trick -> e_sb fp8e5 (queue of 33)
                A(it-1,g): DoubleRow fp8 apply acc += [v|1]^T e
                g==0: conv_chunk(it-2) (9 MMs K=96 + fused wo@w_1)
    normalize(it-1): recip bit-trick + gpsimd broadcast; z = acc*r + x
    stage zr dy-shifted conv planes (DMA)

Host folds: G=Wq^T Wk into one [65,33] projection, biases via ones
rows, gamma into wv, bn_s into conv weights, wo@w{2,3}_1 fused.
"""

import sys

import ml_dtypes
import numpy as np

for _p in ("/opt/trn_rl_repo", "/root/.axon_site/_ro/trn_rl_repo"):
    if _p not in sys.path:
        sys.path.append(_p)

import concourse.bass as bass
import concourse.mybir as mybir
import concourse.tile as tile
from concourse import bacc
from concourse.bass_utils import run_bass_kernel_spmd

B, C, H, W = 8, 32, 64, 64
CQ = C // 2
HW = H * W
NCORES = 8

IC = 512             # i-chunk width
NCH = HW // IC       # 8 chunks
JT = 128             # j-tile (partitions)
NJT = HW // JT       # 32 j-tiles
NGR = NJT // 2       # 16 granules of 2 j-tiles
PW = W + 2           # padded conv width (66)
PHW = PW * (H + 2)   # padded conv plane
RPC = IC // W        # spatial rows per chunk (8)
KE = 2 * C + 1       # energy contraction, zero-padded to 65 rows
KER = C + 1          # real contraction rows (32 ch + 1 aug)

F32 = mybir.dt.float32
F32R = mybir.dt.float32r
I32 = mybir.dt.int32
I8 = mybir.dt.int8
F8E4 = mybir.dt.float8e4
F8E5 = mybir.dt.float8e5
AF = mybir.ActivationFunctionType
ALU = mybir.AluOpType
DR = mybir.MatmulPerfMode.DoubleRow

# Schraudolph e5m2: i8 = E*4*log2(e) + SCH_C, bitcast -> 2^((i-60)/4)
# ~= e^E * 2^((SCH_C-60)/4).  ACT path matches via exp-input bias
# ln(2)*(SCH_C-60)/4.
SCH_C = 63.0
SCH_SLOPE = float(4.0 / np.log(2.0))
ACT_BIAS = float(np.log(2.0) * (SCH_C - 60.0) / 4.0)
RECIP_MAGIC = 0x7EF312AC

# exp engine schedule per chunk: one entry per granule, 'A'=ACT 'D'=DVE
ESCHED = "ADADADADADADADAD"
assert len(ESCHED) == NGR


def _r(ap):
    return ap.bitcast(F32R)


def build_program():
    nc = bacc.Bacc("TRN2", target_bir_lowering=False, debug=False)

    x2d = nc.dram_tensor("x2", [C, HW], F32R, kind="ExternalInput").ap()
    x3d = nc.dram_tensor("x3", [C, HW], F32R, kind="ExternalInput").ap()
    onesd = nc.dram_tensor("ones", [1, HW], F32R, kind="ExternalInput").ap()
    wpd = nc.dram_tensor("wp", [C * 2 + 1, KE], F32R, kind="ExternalInput").ap()
    wvad = nc.dram_tensor("wva", [C * 2 + 1, 2 * C + 2], F32R, kind="ExternalInput").ap()
    wcsd = nc.dram_tensor("wcs", [3 * C, 6 * C], F32R, kind="ExternalInput").ap()
    bbd = nc.dram_tensor("bb", [2 * C, 1], F32, kind="ExternalInput").ap()
    wabd = nc.dram_tensor("wab", [KE, 2 * C], F32R, kind="ExternalInput").ap()
    bfind = nc.dram_tensor("bfin", [C, 1], F32, kind="ExternalInput").ap()
    outd = nc.dram_tensor("out", [C, HW], F32, kind="ExternalOutput").ap()

    with tile.TileContext(nc) as tc:
        _emit(nc, tc, x2d, x3d, onesd, wpd, wvad, wcsd, bbd,
              wabd, bfind, outd)
    nc.compile()
    return nc


def _emit(nc, tc, x2d, x3d, onesd, wpd, wvad, wcsd, bbd,
          wabd, bfind, outd):
    from contextlib import ExitStack

    ctx = ExitStack()
    with ctx:
        consts = ctx.enter_context(tc.tile_pool(name="consts", bufs=1))
        xp = ctx.enter_context(tc.tile_pool(name="xp", bufs=1))
        qk = ctx.enter_context(tc.tile_pool(name="qk", bufs=1))
        vs = ctx.enter_context(tc.tile_pool(name="vs", bufs=1))
        es = ctx.enter_context(tc.tile_pool(name="es", bufs=33))
        zs = ctx.enter_context(tc.tile_pool(name="zs", bufs=2))
        outp = ctx.enter_context(tc.tile_pool(name="outp", bufs=2))
        zrp = ctx.enter_context(tc.tile_pool(name="zrp", bufs=1))
        ep = ctx.enter_context(tc.tile_pool(name="ep", bufs=2, space="PSUM"))
        accp = ctx.enter_context(tc.tile_pool(name="accp", bufs=2, space="PSUM"))
        cvp = ctx.enter_context(tc.tile_pool(name="cvp", bufs=2, space="PSUM"))

        # ---- constant loads -------------------------------------------------
        wp = consts.tile([C * 2 + 1, KE], F32R, tag="wp")
        nc.sync.dma_start(out=wp[:], in_=wpd)
        wva = consts.tile([C * 2 + 1, 2 * C + 2], F32R, tag="wva")
        nc.sync.dma_start(out=wva[:], in_=wvad)
        wcs = consts.tile([3 * C, 6 * C], F32R, tag="wcs")
        nc.sync.dma_start(out=wcs[:], in_=wcsd)
        bb2 = consts.tile([2 * C, 1], F32, tag="bb")
        nc.sync.dma_start(out=bb2[:], in_=bbd)
        wab2 = consts.tile([KE, 2 * C], F32R, tag="wab")
        nc.sync.dma_start(out=wab2[:], in_=wabd)
        bfin = consts.tile([C, 1], F32, tag="bfin")
        nc.sync.dma_start(out=bfin[:], in_=bfind)
        ebias = consts.tile([JT, 1], F32, tag="ebias")
        nc.gpsimd.memset(ebias[:], ACT_BIAS)
        # warm the ACT exp table set while input DMAs run
        tblw = consts.tile([1, 1], F32, tag="tblw")
        nc.scalar.activation(tblw[:], ebias[0:1, 0:1], AF.Exp)

        # ---- inputs (split loads across queues for overlap) ----------------
        HB = HW // 2
        xcat = xp.tile([2 * C + 1, HW], F32R, tag="xcat")
        x3b = xp.tile([C, HW], F32, tag="x3b")
        nc.sync.dma_start(out=xcat[2 * C:2 * C + 1, :], in_=onesd)
        nc.sync.dma_start(out=xcat[0:C, 0:HB], in_=x2d[:, 0:HB])
        nc.sync.dma_start(out=xcat[0:C, HB:HW], in_=x2d[:, HB:HW])
        nc.scalar.dma_start(out=xcat[C:2 * C, 0:HB], in_=x3d[:, 0:HB])
        nc.scalar.dma_start(out=xcat[C:2 * C, HB:HW], in_=x3d[:, HB:HW])
        nc.gpsimd.dma_start(out=x3b[:, 0:HB], in_=x3d.bitcast(F32)[:, 0:HB])
        nc.gpsimd.dma_start(out=x3b[:, HB:HW], in_=x3d.bitcast(F32)[:, HB:HW])

        # ---- P/xm projections into fp8 --------------------------------------
        # p8[a, i] = fp8(W_P^T xcat[:, i]) : [33, HW]
        # xm8[a, j] = fp8([x2*x3; 1][:, j]) : [33, HW]
        # normal-mode K=65 zero-padded layout.  The whole main-loop PE
        # stream stays in ONE weight-path mode (normal, K>=65): mixing
        # DoubleRow and normal matmuls, or K<=64 rows, drops the PE to
        # half rate on HW.
        p8 = qk.tile([KE, HW], F8E4, tag="p8")
        xm8 = qk.tile([KE, HW], F8E4, tag="xm8")
        nc.gpsimd.memset(xm8[C:2 * C, :], 0.0)
        nc.gpsimd.memset(xm8[2 * C:KE, :], 0.0)
        nc.gpsimd.memset(xm8[C:KER, :], 1.0)

        for blk in range(NCH):
            sl = slice(blk * IC, (blk + 1) * IC)
            nc.vector.tensor_mul(xm8[0:C, sl],
                                 xcat[0:C, sl].bitcast(F32),
                                 x3b[:, sl])
            # projections use the acc-pool PSUM (idle in the prologue) so
            # iteration 0's energies don't serialize behind them in the
            # shared e_ps buffer cycle
            pp = accp.tile([2 * C + 2, IC], F32, tag="acc", name="pp")
            nc.tensor.matmul(pp[0:KE, 0:IC], wp[:], xcat[:, sl],
                             start=True, stop=True)
            nc.scalar.activation(p8[:, sl], pp[0:KE, 0:IC], AF.Copy)

        # ---- v^T stack: vstk[j_local, jt, c] = v2|v3|ones (fp8e4) ----------
        VM = 2 * C + 2   # 66 v columns (v2|v3|ones|pad)
        VST = 80         # j-tile stride in vstk (%16==0 for dual-fp8 LW)
        vstk = vs.tile([JT, NJT, VST], F8E4, tag="vstk")
        nc.gpsimd.memset(vstk[:, :, VM:VST], 0.0)
        VPB = 7  # v-tiles per psum half-bank

        def emit_vstack(jt, n_here):
            vt = ep.tile([JT, 2 * IC], F32, tag="e")
            for t in range(n_here):
                off = (t // VPB) * IC + (t % VPB) * VM
                nc.tensor.matmul(
                    vt[:, off:off + VM],
                    xcat[:, (jt + t) * JT:(jt + t + 1) * JT],
                    wva[:],
                    start=True, stop=True,
                )
            for half in range(2):
                cnt = min(VPB, n_here - half * VPB)
                if cnt <= 0:
                    break
                nc.scalar.activation(
                    vstk[:, jt + half * VPB:jt + half * VPB + cnt, 0:VM],
                    vt[:, half * IC:half * IC + cnt * VM]
                    .rearrange("p (t c) -> p t c", c=VM),
                    AF.Copy,
                )

        # static rstk, contraction rows C..KE zero-padded once
        rstk = vs.tile([KE, 2, IC], F32R, tag="rstk")
        nc.gpsimd.memset(rstk[C:2 * C, :, :].bitcast(F32), 0.0)
        nc.gpsimd.memset(rstk[2 * C:KE, :, :].bitcast(F32), 0.0)

        # ---- padded conv plane: z2 rows 0-31, z3 rows 32-63 ----------------
        z23p = zs.tile([2 * C, PHW], F32R, tag="z23p")
        z3d_ = z23p.rearrange("p (h w) -> p h w", h=H + 2, w=PW)
        nc.gpsimd.memset(z3d_[:, 0:1, :].bitcast(F32), 0.0)
        nc.gpsimd.memset(z3d_[:, H + 1:H + 2, :].bitcast(F32), 0.0)
        nc.gpsimd.memset(z3d_[:, 1:H + 1, 0:1].bitcast(F32), 0.0)
        nc.gpsimd.memset(z3d_[:, 1:H + 1, PW - 1:PW].bitcast(F32), 0.0)

        # ---- zR staging: 3 dy-shifted copies of the z plane rows -----------
        ZRL = 544
        zr2 = zrp.tile([3 * C, 2, ZRL], F32R, tag="zr2")
        zr3 = zrp.tile([3 * C, 2, ZRL], F32R, tag="zr3")

        def emit_zr(n, gs):
            src0 = RPC * n * PW
            for g in gs:
                eng = nc.sync if g % 2 == 0 else nc.gpsimd
                eng.dma_start(
                    out=zr2[32 * g:32 * g + C, n % 2, 0:RPC * PW],
                    in_=z23p[0:C, src0 + g * PW:src0 + g * PW + RPC * PW])
                eng.dma_start(
                    out=zr3[32 * g:32 * g + C, n % 2, 0:RPC * PW],
                    in_=z23p[C:2 * C, src0 + g * PW:src0 + g * PW + RPC * PW])

        def emit_zr2_partial(n):
            # band g=2 rows 0..6 (plane rows 8n+2 .. 8n+8): ready right
            # after normalize(n)
            s0 = (RPC * n + 2) * PW
            nc.sync.dma_start(out=zr2[64:64 + C, n % 2, 0:7 * PW],
                              in_=z23p[0:C, s0:s0 + 7 * PW])
            nc.gpsimd.dma_start(out=zr3[64:64 + C, n % 2, 0:7 * PW],
                                in_=z23p[C:2 * C, s0:s0 + 7 * PW])

        def emit_zr2_topup(n):
            # band g=2 last row (plane row 8n+9): needs normalize(n+1)
            s0 = (RPC * n + 9) * PW
            nc.sync.dma_start(out=zr2[64:64 + C, n % 2, 7 * PW:8 * PW],
                              in_=z23p[0:C, s0:s0 + PW])
            nc.gpsimd.dma_start(out=zr3[64:64 + C, n % 2, 7 * PW:8 * PW],
                                in_=z23p[C:2 * C, s0:s0 + PW])

        # ---- conv of chunk n (needs zr slices staged) ----------------------
        def conv_chunk(n):
            wb = n % 2
            cps2 = cvp.tile([C, IC], F32, tag="cv")
            cps3 = cvp.tile([C, IC], F32, tag="cv")
            for br, (zr, cp) in enumerate(((zr2, cps2), (zr3, cps3))):
                for dx in range(3):
                    nc.tensor.matmul(
                        cp[:],
                        wcs[:, (br * 3 + dx) * C:(br * 3 + dx + 1) * C],
                        zr[:, wb, dx:dx + RPC * PW]
                        .rearrange("p (r w) -> p r w", r=RPC, w=PW)[:, :, 0:W],
                        start=(dx == 0), stop=(dx == 2),
                    )
            for br, cp in enumerate((cps2, cps3)):
                # relu(conv + bn_b) on ACT (bias is per-partition)
                nc.scalar.activation(
                    rstk[0:C, br, :],
                    cp[:],
                    AF.Relu, bias=bb2[br * C:(br + 1) * C, 0:1],
                )
            op = cvp.tile([C, IC], F32, tag="cv")
            for br in range(2):
                nc.tensor.matmul(op[:], wab2[:, br * C:(br + 1) * C],
                                 rstk[:, br, :],
                                 start=(br == 0), stop=(br == 1))
            ob = outp.tile([C, IC], F32, tag="ob")
            nc.scalar.activation(ob[:], op[:], AF.Identity,
                                 bias=bfin[:, 0:1])
            nc.sync.dma_start(out=outd[:, n * IC:(n + 1) * IC], in_=ob[:])

        # ---- normalize chunk n: z = acc/denom + x --------------------------
        def normalize_cols(n, acc, c0, c1, tag):
            w = c1 - c0
            nr = w // W
            r_sb = zs.tile([1, IC], I32, tag="r7_" + tag, name="r7")
            nc.vector.tensor_scalar(
                out=r_sb[:, 0:w], in0=acc[2 * C:2 * C + 1, c0:c1].bitcast(I32),
                scalar1=-1, scalar2=RECIP_MAGIC,
                op0=ALU.mult, op1=ALU.add,
            )
            rbc = zs.tile([2 * C, IC], F32, tag="rb7_" + tag, name="rb7")
            nc.gpsimd.partition_broadcast(rbc[:, 0:w], r_sb.bitcast(F32)[:, 0:w])
            z23t = zs.tile([2 * C, IC], F32, tag="z7_" + tag, name="z7")
            nc.vector.tensor_mul(z23t[:, 0:w], acc[0:2 * C, c0:c1], rbc[:, 0:w])
            r0 = RPC * n + c0 // W
            nc.vector.tensor_add(
                z3d_[0:2 * C, 1 + r0:1 + r0 + nr, 1:1 + W],
                z23t[:, 0:w].rearrange("p (a b) -> p a b", a=nr, b=W),
                xcat[0:2 * C, n * IC + c0:n * IC + c1].bitcast(F32)
                .rearrange("p (a b) -> p a b", a=nr, b=W),
            )

        def normalize(n, acc):
            r_sb = zs.tile([1, IC], I32, tag="r_sb")
            nc.vector.tensor_scalar(
                out=r_sb[:], in0=acc[2 * C:2 * C + 1, :].bitcast(I32),
                scalar1=-1, scalar2=RECIP_MAGIC,
                op0=ALU.mult, op1=ALU.add,
            )
            rbc = zs.tile([2 * C, IC], F32, tag="rbc")
            nc.gpsimd.partition_broadcast(rbc[:], r_sb.bitcast(F32)[:])
            z23t = zs.tile([2 * C, IC], F32, tag="z23t")
            nc.vector.tensor_mul(z23t[:], acc[0:2 * C, :], rbc[:])
            r0 = RPC * n
            nc.vector.tensor_add(
                z3d_[0:2 * C, 1 + r0:1 + r0 + RPC, 1:1 + W],
                z23t[:, :].rearrange("p (a b) -> p a b", a=RPC, b=W),
                xcat[0:2 * C, n * IC:(n + 1) * IC].bitcast(F32)
                .rearrange("p (a b) -> p a b", a=RPC, b=W),
            )

        # ---- main loop: E/exp(it) + apply(it-1) + conv(it-2) ---------------
        esq = {}          # (chunk, granule) -> e_sb tile
        accs = {}         # chunk -> acc psum tile
        for it in range(NCH + 1):
            if it < NCH:
                accs[it] = accp.tile([2 * C + 2, IC], F32, tag="acc", name="acc")
            i0 = it * IC
            for g in range(NGR):
                # energies FIRST: at exp(g-2) completion the highest-
                # priority ready PE work must be E(g), so exp(g) issues
                # after only 432ns; the applies then fill the exp window
                if it < NCH:
                    e_ps = ep.tile([JT, 2 * IC], F32, tag="e")
                    for t in range(2):
                        jt_ = 2 * g + t
                        nc.tensor.matmul(
                            e_ps[:, t * IC:(t + 1) * IC],
                            xm8[:, jt_ * JT:(jt_ + 1) * JT],
                            p8[:, i0:i0 + IC],
                            start=True, stop=True,
                        )
                    e_sb = es.tile([JT, 2, IC], F8E5, tag="esb")
                    if ESCHED[g] == 'A':
                        nc.scalar.activation(
                            e_sb[:, :, :],
                            e_ps[:].rearrange("p (t n) -> p t n", t=2),
                            AF.Exp, bias=ebias[:, 0:1],
                        )
                    else:
                        nc.vector.tensor_scalar(
                            out=e_sb.bitcast(I8)[:, :, :],
                            in0=e_ps[:].rearrange("p (t n) -> p t n", t=2),
                            scalar1=SCH_SLOPE, scalar2=SCH_C,
                            op0=ALU.mult, op1=ALU.add,
                        )
                    esq[(it, g)] = e_sb
                if it == 0 and g in (3, 7, 11):
                    jt0 = {3: 0, 7: 14, 11: 28}[g]
                    emit_vstack(jt0, min(14, NJT - jt0))
            for g in range(NGR):
                if it >= 1:
                    e_prev = esq.pop((it - 1, g))
                    for t in range(2):
                        nc.tensor.matmul(
                            accs[it - 1][:],
                            vstk[:, 2 * g + t, 0:VM],
                            e_prev[:, t, :],
                            start=(g == 0 and t == 0),
                            stop=(g == NGR - 1 and t == 1),
                        )
            if 1 <= it < NCH:
                normalize(it - 1, accs.pop(it - 1))
                emit_zr(it - 1, (0, 1))
                emit_zr2_partial(it - 1)
                if it >= 2:
                    emit_zr2_topup(it - 2)
                    conv_chunk(it - 2)
        # tail: only chunk 7's normalize + convs 6,7 remain.  Normalize
        # the first spatial row of chunk 7 first so conv(6)'s topup (one
        # plane row) and matmuls can start while the rest normalizes.
        acc7 = accs.pop(NCH - 1)
        for dmy in range(14):
            e_ps = ep.tile([JT, 2 * IC], F32, tag="e")
            nc.tensor.matmul(e_ps[:, 0:IC],
                             xm8[:, (dmy % 8) * JT:(dmy % 8) * JT + JT],
                             p8[:, 0:IC], start=True, stop=True)
        normalize_cols(NCH - 1, acc7, 0, W, "a")
        s0 = (RPC * (NCH - 2) + 9) * PW
        nc.scalar.dma_start(out=zr2[64:64 + C, (NCH - 2) % 2, 7 * PW:8 * PW],
                            in_=z23p[0:C, s0:s0 + PW])
        nc.scalar.dma_start(out=zr3[64:64 + C, (NCH - 2) % 2, 7 * PW:8 * PW],
                            in_=z23p[C:2 * C, s0:s0 + PW])
        conv_chunk(NCH - 2)
        normalize_cols(NCH - 1, acc7, W, IC, "b")
        emit_zr(NCH - 1, (0, 1))
        emit_zr2_partial(NCH - 1)
        emit_zr2_topup(NCH - 1)
        conv_chunk(NCH - 1)


def prepare_params(wq, bq, wk, bk, wv2, bv2, wv3, bv3, gamma2, gamma3,
                   w2_3, bn2_s, bn2_b, w2_1, b2_1,
                   w3_3, bn3_s, bn3_b, w3_1, b3_1, wo, bo):
    f = np.float32
    wq, bq, wk, bk = (np.asarray(a, f) for a in (wq, bq, wk, bk))
    wv2, bv2, wv3, bv3 = (np.asarray(a, f) for a in (wv2, bv2, wv3, bv3))
    w2_3, bn2_s, bn2_b = (np.asarray(a, f) for a in (w2_3, bn2_s, bn2_b))
    w3_3, bn3_s, bn3_b = (np.asarray(a, f) for a in (w3_3, bn3_s, bn3_b))
    w2_1, b2_1, w3_1, b3_1 = (np.asarray(a, f) for a in (w2_1, b2_1, w3_1, b3_1))
    wo, bo = np.asarray(wo, f), np.asarray(bo, f)
    g2 = f(np.asarray(gamma2).reshape(-1)[0])
    g3 = f(np.asarray(gamma3).reshape(-1)[0])

    # energy fold: E[i,j] = P_aug[:,i] . xm_aug[:,j]
    #   P_aug = W_P^T @ [x2; x3; 1],  xm_aug = [x2*x3; 1]
    A = wq.T @ wk                    # [32, 32]
    u = wk.T @ bq                    # [32]
    v = wq.T @ bk                    # [32]
    s = float(bq @ bk)
    wp = np.zeros((2 * C + 1, KE), f)   # cols C+1..KE stay zero (K pad)
    wp[0:C, 0:C] = A
    wp[C:2 * C, 0:C] = A
    wp[2 * C, 0:C] = u
    wp[0:C, C] = v
    wp[C:2 * C, C] = v
    wp[2 * C, C] = s

    wva = np.zeros((2 * C + 1, 2 * C + 2), f)
    wva[0:C, 0:C] = wv2.T * g2
    wva[C:2 * C, C:2 * C] = wv3.T * g3
    wva[2 * C, 0:C] = bv2 * g2
    wva[2 * C, C:2 * C] = bv3 * g3
    wva[2 * C, 2 * C] = 1.0

    wcs = np.zeros((3 * C, 6 * C), f)
    for (br, w3x3, bns) in ((0, w2_3, bn2_s), (1, w3_3, bn3_s)):
        ws = w3x3 * bns[:, None, None, None]  # [o, ci, dy, dx]
        for dy in range(3):
            for dx in range(3):
                wcs[C * dy:C * (dy + 1),
                    (br * 3 + dx) * C:(br * 3 + dx + 1) * C] = \
                    ws[:, :, dy, dx].T

    bb = np.concatenate([bn2_b, bn3_b]).reshape(2 * C, 1).astype(f)
    wab = np.zeros((KE, 2 * C), f)      # rows C..KE stay zero (K pad)
    wab[0:C, 0:C] = (wo @ w2_1).T
    wab[0:C, C:] = (wo @ w3_1).T
    bfin = (wo @ (b2_1 + b3_1) + bo).astype(f).reshape(C, 1)

    return {
        "wp": wp, "wva": wva, "wcs": wcs, "bb": bb,
        "wab": wab, "bfin": bfin,
        "ones": np.ones((1, HW), f),
    }


_CACHED = {}


def _get_program():
    if "nc" not in _CACHED:
        _CACHED["nc"] = build_program()
    return _CACHED["nc"]


def make_in_maps(x2, x3, params):
    x2 = np.ascontiguousarray(np.asarray(x2, np.float32).reshape(B, C, HW))
    x3 = np.ascontiguousarray(np.asarray(x3, np.float32).reshape(B, C, HW))
    return [
        {"x2": x2[b], "x3": x3[b], **params}
        for b in range(NCORES)
    ]


def kernel(x2, x3, **kw):
    params = prepare_params(**kw)
    nc = _get_program()
    in_maps = make_in_maps(x2, x3, params)
    res = run_bass_kernel_spmd(nc, in_maps, list(range(NCORES)))
    out = np.stack([res.results[b]["out"].reshape(C, H, W)
                    for b in range(NCORES)])
    return out.astype(np.float32)


def _ensure_ntff_hook():
    import contextlib
    import ctypes
    import types

    if "antenv.axon_hooks" in sys.modules:
        return
    so_path = "/opt/axon/libaxon_pjrt.so"
    lib = ctypes.CDLL(so_path)
    lib.axon_start_nrt_profile.argtypes = [
        ctypes.POINTER(ctypes.c_int64), ctypes.c_size_t]
    lib.axon_start_nrt_profile.restype = ctypes.c_int64
    lib.axon_stop_nrt_profile.argtypes = [ctypes.c_char_p]
    lib.axon_stop_nrt_profile.restype = ctypes.c_int64

    @contextlib.contextmanager
    def _hook(output_dir, device_ids):
        import jax
        jax.devices()
        if device_ids:
            ids = (ctypes.c_int64 * len(device_ids))(*device_ids)
            rc = lib.axon_start_nrt_profile(ids, len(device_ids))
        else:
            rc = lib.axon_start_nrt_profile(None, 0)
        if rc != 0:
            raise RuntimeError(f"axon_start_nrt_profile rc={rc}")
        try:
            yield
        finally:
            n = lib.axon_stop_nrt_profile(str(output_dir).encode())
            if n < 0:
                raise RuntimeError(f"axon_stop_nrt_profile rc={n}")
            if n == 0:
                print("WARNING: NTFF capture wrote 0 files")

    mod = types.ModuleType("antenv.axon_hooks")
    mod.get_axon_ntff_profile_hook = lambda: _hook
    mod.set_axon_ntff_profile_hook = lambda h: None
    sys.modules["antenv.axon_hooks"] = mod


def run_traced(x2, x3, trace_cores=None, **kw):
    _ensure_ntff_hook()
    params = prepare_params(**kw)
    nc = _get_program()
    in_maps = make_in_maps(x2, x3, params)
    res = run_bass_kernel_spmd(nc, in_maps, list(range(NCORES)),
                               trace=True, trace_cores=trace_cores)
    out = np.stack([res.results[b]["out"].reshape(C, H, W)
                    for b in range(NCORES)])
    return out.astype(np.float32), res
